# revision 1
# baseline (speedup 1.0000x reference)
"""GCN (2x GCNConv + graph-layernorm + prelu + mean-pool + MLP head) on 8 trn2 cores.

Strategy (dst-sharded graph parallel):
  - nodes (and their incoming edges) sharded 8 ways by dst; weight matrices replicated.
  - per-edge gather of source features via gpsimd dma_gather (fp16 table rows),
    segment-sum via one-hot selection matmuls on the TensorEngine (PSUM accumulate).
  - layer-1 aggregates the 2-channel scaled positions (out1 = (D^-1/2 (A+I) D^-1/2 pos) @ W1).
  - global layernorm stats via AllReduce; h2 = x1' @ W2 tables AllGathered in fp16.
  - per-graph mean-pool partials combined via AllReduce; MLP head computed redundantly.
All floating point compute happens on device; the host only shards/sorts/pads
integer index metadata and re-lays-out inputs.
"""

import numpy as np

P = 128


def _cfg_tiny():
    return dict(
        N=1900, E0=8000, G=128, IN_C=2, HID=256, OUT=16,
        NCORES=8, SH=256, CH=512, NCHUNK=4, CALL_TILES=4,
    )


def _cfg_full():
    return dict(
        N=100000, E0=3200000, G=512, IN_C=2, HID=256, OUT=16,
        NCORES=8, SH=12544, CH=32768, NCHUNK=4, CALL_TILES=16,
    )


# ----------------------------------------------------------------- host prep

def _wrap_idx(ix):
    """dma_gather idx layout: idx i -> [i%16 + 16k, i//16] for all k (replicated)."""
    m = ix.reshape(-1, 16).T
    return np.tile(m, (8, 1)).astype(np.int16)


def host_prep(cfg, pos, edge_index, batch):
    c = cfg
    N, E0, G, SH, CH, NCH = c["N"], c["E0"], c["G"], c["SH"], c["CH"], c["NCHUNK"]
    NCORES = c["NCORES"]
    NPAD = SH * NCORES
    NT = SH // P                      # dst tiles per core
    src = np.concatenate([edge_index[0], np.arange(N, dtype=np.int64)]).astype(np.int64)
    dst = np.concatenate([edge_index[1], np.arange(N, dtype=np.int64)]).astype(np.int64)
    deg = np.bincount(dst, minlength=NPAD).astype(np.float32)

    so = np.argsort(dst, kind="stable")
    dsts = dst[so]
    srcs = src[so]
    bounds = np.searchsorted(dsts, np.arange(NCORES + 1) * SH)

    percore = []
    counts = np.zeros((NCORES, NCH, NT), np.int64)
    for ci in range(NCORES):
        s = srcs[bounds[ci]:bounds[ci + 1]]
        d = dsts[bounds[ci]:bounds[ci + 1]] - ci * SH
        chunk = s // CH
        o = np.lexsort((d, chunk))
        s, d, chunk = s[o], d[o], chunk[o]
        dt = d // P
        cnt = np.bincount(chunk * NT + dt, minlength=NCH * NT).reshape(NCH, NT)
        counts[ci] = cnt
        percore.append((s, d, chunk, cnt))

    tiles = np.maximum(1, (counts.max(axis=0) + P - 1) // P)   # [NCH, NT] tiles per cell
    tiles_chunk = tiles.sum(axis=1)                            # [NCH]
    TOT = int(tiles.sum())

    idx_streams, dst_streams = [], []
    for ci in range(NCORES):
        s, d, chunk, cnt = percore[ci]
        idxs = np.zeros(TOT * P, np.int16)
        dcol = np.full(TOT * P, 999.0, np.float32)
        # cells laid out chunk-major, dtile-minor; each cell padded to tiles[ch][t]*P
        cell_starts = {}
        off = 0
        eoff = np.concatenate([[0], np.cumsum(cnt.ravel())])
        for ch in range(NCH):
            for t in range(NT):
                n = cnt[ch, t]
                e0 = eoff[ch * NT + t]
                slots = tiles[ch, t] * P
                idxs[off:off + n] = (s[e0:e0 + n] - ch * CH).astype(np.int16)
                dcol[off:off + n] = (d[e0:e0 + n] - t * P).astype(np.float32)
                cell_starts[(ch, t)] = off
                off += slots
        assert off == TOT * P
        idx_streams.append(idxs)
        dst_streams.append(dcol)

    # per-call wrapped idx layout (calls split per chunk, <= CALL_TILES tiles)
    CALLS = []   # list of (chunk, ntiles) in stream order
    for ch in range(NCH):
        rem = int(tiles_chunk[ch])
        while rem > 0:
            k = min(c["CALL_TILES"], rem)
            CALLS.append((ch, k))
            rem -= k
    idx_wrapped = []
    for ci in range(NCORES):
        stream = idx_streams[ci]
        parts, off = [], 0
        for (_ch, k) in CALLS:
            parts.append(_wrap_idx(stream[off:off + k * P]))
            off += k * P
        idx_wrapped.append(np.concatenate(parts, axis=1))      # [128, TOT*8]
    dst_cols = [ds.reshape(TOT, P).T.copy() for ds in dst_streams]  # [128, TOT]

    # pooling metadata
    gbase = np.zeros(NCORES, np.int32)
    batch_local = np.full((NCORES, SH), 999.0, np.float32)
    for ci in range(NCORES):
        lo, hi = ci * SH, min((ci + 1) * SH, N)
        gbase[ci] = batch[lo]
        batch_local[ci, :hi - lo] = (batch[lo:hi] - batch[lo]).astype(np.float32)
        assert batch[hi - 1] - batch[lo] < P - 2, "too many graphs in one shard"
    cnts = np.bincount(batch, minlength=G).astype(np.float32)

    meta = dict(
        NPAD=NPAD, NT=NT, TOT=TOT, tiles=tiles, tiles_chunk=tiles_chunk,
        CALLS=CALLS, NPADROWS=NPAD - N,
    )
    # device-layout inputs (identical shapes across cores; values differ where noted)
    NTF = NPAD // P
    pos_pad = np.zeros((NPAD, c["IN_C"]), np.float32)
    pos_pad[:N] = pos
    pos_dev = pos_pad.reshape(NTF, P, c["IN_C"]).transpose(1, 0, 2).copy()
    deg_dev = deg.reshape(NTF, P).T.copy()
    deg_shard = [deg[ci * SH:(ci + 1) * SH].reshape(NT, P).T.copy() for ci in range(NCORES)]
    batch_dev = [batch_local[ci].reshape(NT, P).T.copy() for ci in range(NCORES)]
    cnt_dev = np.zeros((P, (G + P - 1) // P), np.float32)
    for g in range(G):
        cnt_dev[g % P, g // P] = cnts[g]
    ins = []
    for ci in range(NCORES):
        ins.append(dict(
            pos_dev=pos_dev, deg_dev=deg_dev, deg_shard=deg_shard[ci],
            idxs=idx_wrapped[ci], dstc=dst_cols[ci],
            batchl=batch_dev[ci], cntg=cnt_dev,
            gbase=np.array([[float(gbase[ci])]], np.float32),
        ))
    return meta, ins


def _prep_weights(cfg, W):
    """Re-layout weights for device (pure replication / transpose-free reshapes)."""
    c = cfg
    HID, OUT, IN_C, G = c["HID"], c["OUT"], c["IN_C"], c["G"]
    NH = HID // P                      # channel halves (2)
    w = {}
    w["w1"] = W["w_conv1"].astype(np.float32)                        # [2, 256]
    w["b1_cols"] = W["b_conv1"].reshape(NH, P).T.copy()              # [128, NH]
    w["ln1w_cols"] = W["ln1_w"].reshape(NH, P).T.copy()
    w["ln1b_cols"] = W["ln1_b"].reshape(NH, P).T.copy()
    w["w2_kt"] = np.ascontiguousarray(W["w_conv2"].reshape(NH, P, HID).transpose(1, 0, 2))  # [128, NH, 256]
    w["b2_bc"] = np.tile(W["b_conv2"][None, :], (P, 1)).astype(np.float32)   # [128, 256]
    w["ln2w_bc"] = np.tile(W["ln2_w"][None, :], (P, 1)).astype(np.float32)
    w["ln2b_bc"] = np.tile(W["ln2_b"][None, :], (P, 1)).astype(np.float32)
    w["wl1_kt"] = np.ascontiguousarray(W["w_lin1"].reshape(NH, P, HID // 2).transpose(1, 0, 2))  # [128, NH, 128]
    w["bl1_bc"] = np.tile(W["b_lin1"][None, :], (P, 1)).astype(np.float32)   # [128, 128]
    w["lnmw_bc"] = np.tile(W["lnm_w"][None, :], (P, 1)).astype(np.float32)
    w["lnmb_bc"] = np.tile(W["lnm_b"][None, :], (P, 1)).astype(np.float32)
    w["wl2"] = W["w_lin2"].astype(np.float32)                        # [128, 16]
    w["bl2_bc"] = np.tile(W["b_lin2"][None, :], (P, 1)).astype(np.float32)   # [128, 16]
    w["a1"] = float(W["a1"]); w["a2"] = float(W["a2"]); w["am"] = float(W["am"])
    return w


# ----------------------------------------------------------------- device build

def build_program(cfg, meta, weights):
    import concourse.bass as bass
    import concourse.mybir as mybir
    import concourse.tile as tile
    from concourse import bacc
    from concourse.masks import make_identity

    c = cfg
    dt = mybir.dt
    N, G, HID, OUT, IN_C = c["N"], c["G"], c["HID"], c["OUT"], c["IN_C"]
    SH, CH, NCH = c["SH"], c["CH"], c["NCHUNK"]
    NCORES = c["NCORES"]
    NPAD, NT, TOT = meta["NPAD"], meta["NT"], meta["TOT"]
    tiles, CALLS = meta["tiles"], meta["CALLS"]
    NTF = NPAD // P
    NH = HID // P
    GT = (G + P - 1) // P              # graph tiles (4)
    NPADROWS = meta["NPADROWS"]
    EPS = 1e-5
    CORE_IDS = list(range(NCORES))
    f32, f16, i16, i32 = dt.float32, dt.float16, dt.int16, dt.int32
    AF = mybir.ActivationFunctionType
    OP = mybir.AluOpType

    nc = bacc.Bacc("TRN2", debug=False, num_devices=NCORES, num_swdge_queues=4)

    # ---- I/O ----
    pos_in = nc.declare_dram_parameter("pos_dev", [P, NTF, IN_C], f32, isOutput=False)
    deg_in = nc.declare_dram_parameter("deg_dev", [P, NTF], f32, isOutput=False)
    degs_in = nc.declare_dram_parameter("deg_shard", [P, NT], f32, isOutput=False)
    idx_in = nc.declare_dram_parameter("idxs", [P, TOT * 8], i16, isOutput=False)
    dstc_in = nc.declare_dram_parameter("dstc", [P, TOT], f32, isOutput=False)
    batch_in = nc.declare_dram_parameter("batchl", [P, NT], f32, isOutput=False)
    cnt_in = nc.declare_dram_parameter("cntg", [P, GT], f32, isOutput=False)
    gbase_in = nc.declare_dram_parameter("gbase", [1, 1], f32, isOutput=False)
    wt = {}
    wspec = dict(
        w1=[IN_C, HID], b1_cols=[P, NH], ln1w_cols=[P, NH], ln1b_cols=[P, NH],
        w2_kt=[P, NH, HID], b2_bc=[P, HID], ln2w_bc=[P, HID], ln2b_bc=[P, HID],
        wl1_kt=[P, NH, HID // 2], bl1_bc=[P, HID // 2], lnmw_bc=[P, HID // 2],
        lnmb_bc=[P, HID // 2], wl2=[HID // 2, OUT], bl2_bc=[P, OUT],
    )
    for k, shp in wspec.items():
        wt[k] = nc.declare_dram_parameter(k, shp, f32, isOutput=False)
    out_ext = nc.declare_dram_parameter("out", [G, OUT], f32, isOutput=True)

    # ---- internal DRAM ----
    qtab = nc.dram_tensor("qtab", [NPAD, P], f16)                 # L1 table (2 real cols)
    gshard = nc.dram_tensor("gshard", [SH, HID], f16)             # local h2*dinv
    gtab = nc.dram_tensor("gtab", [NPAD, HID], f16, addr_space="Shared")
    st1_in = nc.dram_tensor("st1_in", [1, P], f32)
    st1_out = nc.dram_tensor("st1_out", [1, P], f32, addr_space="Shared")
    st2_in = nc.dram_tensor("st2_in", [1, P], f32)
    st2_out = nc.dram_tensor("st2_out", [1, P], f32, addr_space="Shared")
    POOLR = (GT + 1) * P                                          # 640 rows
    pool_in = nc.dram_tensor("pool_in", [POOLR, HID], f32)
    pool_out = nc.dram_tensor("pool_out", [POOLR, HID], f32, addr_space="Shared")

    a1, a2, am = weights["a1"], weights["a2"], weights["am"]

    with tile.TileContext(nc) as tc:
        with tc.tile_pool(name="persist", bufs=1) as pp, \
             tc.tile_pool(name="psc", bufs=2, space="PSUM") as psc:
            # ---- persistent small tiles ----
            iota_i = pp.tile([P, P], i32)
            nc.gpsimd.iota(iota_i[:], pattern=[[1, P]], base=0, channel_multiplier=0)
            iota_h = pp.tile([P, P], f16)
            nc.vector.tensor_copy(out=iota_h[:], in_=iota_i[:])
            iota_f = pp.tile([P, P], f32)
            nc.vector.tensor_copy(out=iota_f[:], in_=iota_i[:])
            ident = pp.tile([P, P], f32)
            make_identity(nc, ident[:])
            ones_col = pp.tile([P, 1], f32)
            nc.vector.memset(ones_col[:], 1.0)
            ones_row = pp.tile([1, P], f32)
            nc.vector.memset(ones_row[:], 1.0)

            dstc_sb = pp.tile([P, TOT], f32)
            nc.sync.dma_start(out=dstc_sb[:], in_=dstc_in[:])
            dstc16 = pp.tile([P, TOT], f16)
            nc.vector.tensor_copy(out=dstc16[:], in_=dstc_sb[:])

            wsb = {}
            for k, shp in wspec.items():
                wsb[k] = pp.tile(shp, f32, name=f"w_{k}")
                nc.sync.dma_start(out=wsb[k][:], in_=wt[k][:])

            # dinv (full + shard)
            deg_f = pp.tile([P, NTF], f32)
            nc.sync.dma_start(out=deg_f[:], in_=deg_in[:])
            nc.vector.tensor_scalar(out=deg_f[:], in0=deg_f[:], scalar1=1.0,
                                    scalar2=None, op0=OP.max)
            nc.scalar.sqrt(deg_f[:], deg_f[:])
            dinv_f = pp.tile([P, NTF], f32)
            nc.vector.reciprocal(dinv_f[:], deg_f[:])
            deg_s = pp.tile([P, NT], f32)
            nc.sync.dma_start(out=deg_s[:], in_=degs_in[:])
            nc.vector.tensor_scalar(out=deg_s[:], in0=deg_s[:], scalar1=1.0,
                                    scalar2=None, op0=OP.max)
            nc.scalar.sqrt(deg_s[:], deg_s[:])
            dinv_s = pp.tile([P, NT], f32)
            nc.vector.reciprocal(dinv_s[:], deg_s[:])

            # ---- build q table: q = pos * dinv (fp16 rows of qtab) ----
            with tc.tile_pool(name="p0", bufs=1) as p0:
                pos_sb = p0.tile([P, NTF, IN_C], f32)
                nc.sync.dma_start(out=pos_sb[:], in_=pos_in[:])
                q16 = p0.tile([P, NTF, IN_C], f16)
                for ch in range(IN_C):
                    nc.vector.tensor_tensor(out=q16[:, :, ch], in0=pos_sb[:, :, ch],
                                            in1=dinv_f[:], op=OP.mult)
                qtab_v = qtab[:].rearrange("(a b) d -> b a d", b=P)
                nc.sync.dma_start(out=qtab_v[:, :, 0:IN_C], in_=q16[:])

            # helper: cross-partition scalar sum -> [1,1] psum -> sbuf tile
            def part_sum(src_col, w_):
                ps = psc.tile([1, src_col.shape[1]], f32, space="PSUM", tag="psc_scratch")
                nc.tensor.matmul(out=ps[:], lhsT=ones_col[:], rhs=src_col[:],
                                 start=True, stop=True)
                dstt = w_.tile([1, src_col.shape[1]], f32, tag="psum_scalar")
                nc.vector.tensor_copy(out=dstt[:], in_=ps[:])
                return dstt

            def bcast_col(vals_row, w_):
                """vals_row [1, k] -> [128, k] replicated."""
                k = vals_row.shape[1]
                ps = psc.tile([P, k], f32, space="PSUM", tag="psc_scratch")
                nc.tensor.matmul(out=ps[:], lhsT=ones_row[:], rhs=vals_row[:],
                                 start=True, stop=True)
                o = w_.tile([P, k], f32, tag="bcast_col")
                nc.vector.tensor_copy(out=o[:], in_=ps[:])
                return o

            # ============ gather + segsum pass (shared for L1 / L2) ============
            def agg_pass(table, elem, rhs_w, accum, acc_w, stage_pool, idxp, selp, psp):
                """table: dram [rows, elem] f16; accum [128, NT, rhs_w] f32 zeroed."""
                gt_tile = 0        # global tile index
                call_i = 0
                off_cols = 0       # idx column offset (8 per tile)
                for chk in range(NCH):
                    order = [t for t in range(NT) for _ in range(int(tiles[chk, t]))]
                    # call partitioning for this chunk
                    pos_in_chunk = 0
                    chunk_tiles = int(tiles[chk].sum())
                    while pos_in_chunk < chunk_tiles:
                        k = min(c["CALL_TILES"], chunk_tiles - pos_in_chunk)
                        idx_t = idxp.tile([P, c["CALL_TILES"] * 8], i16, tag="idx")
                        nc.sync.dma_start(
                            out=idx_t[:, 0:k * 8],
                            in_=idx_in[:, off_cols:off_cols + k * 8])
                        gbuf = stage_pool.tile([P, c["CALL_TILES"], elem], f16, tag="g")
                        nc.gpsimd.dma_gather(
                            out_ap=gbuf[:, 0:k, :],
                            in_ap=table[chk * CH:min((chk + 1) * CH, NPAD), :],
                            idxs_ap=idx_t[:, 0:k * 8],
                            num_idxs=k * P, num_idxs_reg=k * P,
                            elem_size=elem, single_packet=False,
                            queue_num=call_i % 4)
                        # batched one-hot build for the whole call (one DVE op)
                        sel_call = selp.tile([P, c["CALL_TILES"], P], f16, tag="sel")
                        _i = iota_h[:]
                        _db = dstc16[:, gt_tile:gt_tile + k]
                        iota_bc = bass.AP(_i.tensor, _i.offset,
                                          [list(_i.ap[0]), [0, k], list(_i.ap[1])])
                        dst_bc = bass.AP(_db.tensor, _db.offset,
                                         [list(_db.ap[0]), list(_db.ap[1]), [0, P]])
                        nc.vector.tensor_tensor(out=sel_call[:, 0:k, :], in0=iota_bc,
                                                in1=dst_bc, op=OP.is_equal)
                        # consume the k tiles of this call
                        for j in range(k):
                            t = order[pos_in_chunk + j]
                            first = (order[pos_in_chunk + j - 1] != t) if (pos_in_chunk + j) > 0 else True
                            # new psum group when dtile changes (cells are contiguous)
                            if first:
                                ps = psp.tile([P, rhs_w], f32, space="PSUM", tag="pacc")
                            last = (pos_in_chunk + j == chunk_tiles - 1) or \
                                   (order[pos_in_chunk + j + 1] != t)
                            nc.tensor.matmul(
                                out=ps[:], lhsT=sel_call[:, j, :],
                                rhs=gbuf[:, j, 0:rhs_w],
                                start=first, stop=last)
                            if last:
                                nc.vector.tensor_add(out=accum[:, t, :],
                                                     in0=accum[:, t, :], in1=ps[:])
                            gt_tile += 1
                        pos_in_chunk += k
                        off_cols += k * 8
                        call_i += 1

            # =============================== L1 ===============================
            with tc.tile_pool(name="acc1", bufs=1) as a1p:
                accum1 = a1p.tile([P, NT, IN_C], f32)
                nc.vector.memset(accum1[:], 0.0)
                with tc.tile_pool(name="st1", bufs=5) as sp1, \
                     tc.tile_pool(name="idx1", bufs=4) as ip1, \
                     tc.tile_pool(name="sel1", bufs=3) as sl1, \
                     tc.tile_pool(name="ps1", bufs=2, space="PSUM") as pp1, \
                     nc.named_scope("L1agg"):
                    agg_pass(qtab, P, IN_C, accum1, a1p, sp1, ip1, sl1, pp1)

                # ---- x1T = W1 @ (dinv*agg).T ; layout [128ch, NH, SH] ----
                with tc.tile_pool(name="x1", bufs=1) as x1p, nc.named_scope("mid"):
                    x1t = x1p.tile([P, NH, SH], f32)
                    with tc.tile_pool(name="w2w", bufs=4) as wk:
                        for t in range(NT):
                            sc = wk.tile([P, IN_C], f32, tag="sc")
                            nc.vector.tensor_scalar(
                                out=sc[:], in0=accum1[:, t, :],
                                scalar1=dinv_s[:, t:t + 1], scalar2=None, op0=OP.mult)
                            pt = psc.tile([IN_C, P], f32, space="PSUM", tag="psc_scratch")
                            nc.tensor.transpose(out=pt[:], in_=sc[:], identity=ident[:])
                            p1t = wk.tile([IN_C, P], f32, tag="p1t_sb")
                            nc.vector.tensor_copy(out=p1t[:], in_=pt[:])
                            for h in range(NH):
                                psx = psc.tile([P, P], f32, space="PSUM", tag="psc_scratch")
                                nc.tensor.matmul(
                                    out=psx[:], lhsT=wsb["w1"][:, h * P:(h + 1) * P],
                                    rhs=p1t[:], start=True, stop=True)
                                nc.vector.tensor_scalar(
                                    out=x1t[:, h, t * P:(t + 1) * P], in0=psx[:],
                                    scalar1=wsb["b1_cols"][:, h:h + 1], scalar2=None,
                                    op0=OP.add)

                    # ---- ln1 stats (global over x1) ----
                    with tc.tile_pool(name="stats", bufs=1) as stp:
                        s_col = stp.tile([P, 1], f32)
                        nc.vector.tensor_reduce(out=s_col[:], in_=x1t[:].rearrange("p a b -> p (a b)"),
                                                axis=mybir.AxisListType.X, op=OP.add)
                        CHK = 1024
                        nchk = (NH * SH + CHK - 1) // CHK
                        sq_cols = stp.tile([P, nchk], f32)
                        sq_scr = stp.tile([P, CHK], f32)
                        x1flat = x1t[:].rearrange("p a b -> p (a b)")
                        for ck in range(nchk):
                            lo, hi = ck * CHK, min((ck + 1) * CHK, NH * SH)
                            nc.scalar.activation(out=sq_scr[:, 0:hi - lo], in_=x1flat[:, lo:hi],
                                                 func=AF.Square, accum_out=sq_cols[:, ck:ck + 1])
                        sq_col = stp.tile([P, 1], f32)
                        nc.vector.tensor_reduce(out=sq_col[:], in_=sq_cols[:],
                                                axis=mybir.AxisListType.X, op=OP.add)
                        both = stp.tile([P, 2], f32)
                        nc.vector.tensor_copy(out=both[:, 0:1], in_=s_col[:])
                        nc.vector.tensor_copy(out=both[:, 1:2], in_=sq_col[:])
                        tot = part_sum(both, stp)            # [1,2] local (sum, sumsq)
                        # b1 pad-row corrections
                        b1s_c = stp.tile([P, 2], f32)
                        nc.vector.tensor_copy(out=b1s_c[:, 0:1],
                                              in_=wsb["b1_cols"][:, 0:1])
                        nc.scalar.square(b1s_c[:, 1:2], wsb["b1_cols"][:, 0:1])
                        for h in range(1, NH):
                            nc.vector.tensor_add(out=b1s_c[:, 0:1], in0=b1s_c[:, 0:1],
                                                 in1=wsb["b1_cols"][:, h:h + 1])
                            sqh = stp.tile([P, 1], f32, tag="sqh")
                            nc.scalar.square(sqh[:], wsb["b1_cols"][:, h:h + 1])
                            nc.vector.tensor_add(out=b1s_c[:, 1:2], in0=b1s_c[:, 1:2],
                                                 in1=sqh[:])
                        b1tot = part_sum(b1s_c, stp)         # [1,2] (sum b1, sum b1^2)
                        # allreduce
                        arr = stp.tile([1, P], f32)
                        nc.vector.memset(arr[:], 0.0)
                        nc.vector.tensor_copy(out=arr[:, 0:2], in_=tot[:])
                        nc.sync.dma_start(out=st1_in[:], in_=arr[:])
                        nc.gpsimd.collective_compute(
                            "AllReduce", OP.add, replica_groups=[CORE_IDS],
                            ins=[st1_in[:]], outs=[st1_out[:]])
                        arro = stp.tile([1, P], f32)
                        nc.sync.dma_start(out=arro[:], in_=st1_out[:])
                        # corrected stats -> mean, rstd
                        CNT = float(N * HID)
                        cor = stp.tile([1, 2], f32)
                        nc.vector.tensor_scalar(out=cor[:], in0=b1tot[:],
                                                scalar1=-float(NPADROWS), scalar2=None,
                                                op0=OP.mult)
                        nc.vector.tensor_add(out=cor[:], in0=cor[:], in1=arro[:, 0:2])
                        mean_t = stp.tile([1, 1], f32)
                        nc.vector.tensor_scalar(out=mean_t[:], in0=cor[:, 0:1],
                                                scalar1=1.0 / CNT, scalar2=None, op0=OP.mult)
                        ex2 = stp.tile([1, 1], f32)
                        nc.vector.tensor_scalar(out=ex2[:], in0=cor[:, 1:2],
                                                scalar1=1.0 / CNT, scalar2=None, op0=OP.mult)
                        m2 = stp.tile([1, 1], f32)
                        nc.vector.tensor_tensor(out=m2[:], in0=mean_t[:], in1=mean_t[:],
                                                op=OP.mult)
                        var = stp.tile([1, 1], f32)
                        nc.vector.tensor_tensor(out=var[:], in0=ex2[:], in1=m2[:],
                                                op=OP.subtract)
                        nc.scalar.sqrt(var[:], var[:])
                        nc.vector.tensor_scalar(out=var[:], in0=var[:], scalar1=EPS,
                                                scalar2=None, op0=OP.add)
                        rstd = stp.tile([1, 1], f32)
                        nc.vector.reciprocal(rstd[:], var[:])
                        pack = stp.tile([1, 2], f32)
                        nc.vector.tensor_copy(out=pack[:, 0:1], in_=mean_t[:])
                        nc.vector.tensor_copy(out=pack[:, 1:2], in_=rstd[:])
                        mr = bcast_col(pack, stp)            # [128,2] (mean, rstd)
                        # per-channel affine a,c for both halves
                        acol = stp.tile([P, NH], f32)
                        ccol = stp.tile([P, NH], f32)
                        nc.vector.tensor_scalar(out=acol[:], in0=wsb["ln1w_cols"][:],
                                                scalar1=mr[:, 1:2], scalar2=None, op0=OP.mult)
                        nc.vector.tensor_scalar(out=ccol[:], in0=acol[:],
                                                scalar1=mr[:, 0:1], scalar2=None, op0=OP.mult)
                        nc.vector.tensor_tensor(out=ccol[:], in0=wsb["ln1b_cols"][:],
                                                in1=ccol[:], op=OP.subtract)
                        for h in range(NH):
                            nc.vector.tensor_scalar(
                                out=x1t[:, h, :], in0=x1t[:, h, :],
                                scalar1=acol[:, h:h + 1], scalar2=ccol[:, h:h + 1],
                                op0=OP.mult, op1=OP.add)
                        nc.scalar.activation(out=x1t[:].rearrange("p a b -> p (a b)"),
                                             in_=x1t[:].rearrange("p a b -> p (a b)"),
                                             func=AF.Prelu, alpha=a1)

                    # ---- h2 = x1' @ W2 ; g = dinv * h2 -> gshard fp16 ----
                    with tc.tile_pool(name="h2w", bufs=3) as h2w, \
                         tc.tile_pool(name="ph2", bufs=2, space="PSUM") as ph2:
                        for t in range(NT):
                            ps2 = ph2.tile([P, HID], f32, space="PSUM", tag="ph2")
                            for h in range(NH):
                                nc.tensor.matmul(
                                    out=ps2[:], lhsT=x1t[:, h, t * P:(t + 1) * P],
                                    rhs=wsb["w2_kt"][:, h, :], start=(h == 0), stop=(h == NH - 1))
                            g16 = h2w.tile([P, HID], f16, tag="g16")
                            nc.vector.tensor_scalar(
                                out=g16[:], in0=ps2[:],
                                scalar1=dinv_s[:, t:t + 1], scalar2=None, op0=OP.mult)
                            nc.sync.dma_start(out=gshard[t * P:(t + 1) * P, :], in_=g16[:])
            # pools a1p/x1p closed
            with nc.named_scope("allgather"):
                nc.gpsimd.collective_compute(
                    "AllGather", OP.bypass, replica_groups=[CORE_IDS],
                    ins=[gshard[:]], outs=[gtab[:]])

            # =============================== L2 ===============================
            with tc.tile_pool(name="acc2", bufs=1) as a2p:
                accum2 = a2p.tile([P, NT, HID], f32)
                nc.vector.memset(accum2[:].rearrange("p a b -> p (a b)"), 0.0)
                with tc.tile_pool(name="st2", bufs=5) as sp2, \
                     tc.tile_pool(name="idx2", bufs=4) as ip2, \
                     tc.tile_pool(name="sel2", bufs=3) as sl2, \
                     tc.tile_pool(name="ps2", bufs=2, space="PSUM") as pp2, \
                     nc.named_scope("L2agg"):
                    agg_pass(gtab, HID, HID, accum2, a2p, sp2, ip2, sl2, pp2)

                # ---- x2 = dinv*agg + b2 (in place), ln2 stats, prelu ----
                with tc.tile_pool(name="stats2", bufs=1) as stp:
                    _dv = dinv_s[:]
                    dinv_bc = bass.AP(_dv.tensor, _dv.offset,
                                      [list(_dv.ap[0]), list(_dv.ap[1]), [0, HID]])
                    nc.vector.tensor_tensor(out=accum2[:], in0=accum2[:],
                                            in1=dinv_bc, op=OP.mult)
                    _b2 = wsb["b2_bc"][:]
                    b2_bc3 = bass.AP(_b2.tensor, _b2.offset,
                                     [list(_b2.ap[0]), [0, NT], list(_b2.ap[1])])
                    nc.vector.tensor_tensor(out=accum2[:], in0=accum2[:],
                                            in1=b2_bc3, op=OP.add)
                    s_col = stp.tile([P, 1], f32)
                    nc.vector.tensor_reduce(out=s_col[:],
                                            in_=accum2[:].rearrange("p a b -> p (a b)"),
                                            axis=mybir.AxisListType.X, op=OP.add)
                    sq_cols2 = stp.tile([P, NT], f32)
                    sq_scr2 = stp.tile([P, HID], f32)
                    for t in range(NT):
                        nc.scalar.activation(out=sq_scr2[:], in_=accum2[:, t, :],
                                             func=AF.Square, accum_out=sq_cols2[:, t:t + 1])
                    sq_col = stp.tile([P, 1], f32)
                    nc.vector.tensor_reduce(out=sq_col[:], in_=sq_cols2[:],
                                            axis=mybir.AxisListType.X, op=OP.add)
                    both = stp.tile([P, 2], f32)
                    nc.vector.tensor_copy(out=both[:, 0:1], in_=s_col[:])
                    nc.vector.tensor_copy(out=both[:, 1:2], in_=sq_col[:])
                    tot = part_sum(both, stp)
                    # b2 pad corrections (pad rows equal b2 exactly)
                    b2p = stp.tile([1, 2], f32)
                    nc.vector.tensor_reduce(out=b2p[:, 0:1], in_=wsb["b2_bc"][0:1, :],
                                            axis=mybir.AxisListType.X, op=OP.add)
                    b2sq = stp.tile([1, HID], f32)
                    nc.scalar.square(b2sq[:], wsb["b2_bc"][0:1, :])
                    nc.vector.tensor_reduce(out=b2p[:, 1:2], in_=b2sq[:],
                                            axis=mybir.AxisListType.X, op=OP.add)
                    arr = stp.tile([1, P], f32)
                    nc.vector.memset(arr[:], 0.0)
                    nc.vector.tensor_copy(out=arr[:, 0:2], in_=tot[:])
                    nc.sync.dma_start(out=st2_in[:], in_=arr[:])
                    nc.gpsimd.collective_compute(
                        "AllReduce", OP.add, replica_groups=[CORE_IDS],
                        ins=[st2_in[:]], outs=[st2_out[:]])
                    arro = stp.tile([1, P], f32)
                    nc.sync.dma_start(out=arro[:], in_=st2_out[:])
                    CNT = float(N * HID)
                    cor = stp.tile([1, 2], f32)
                    nc.vector.tensor_scalar(out=cor[:], in0=b2p[:],
                                            scalar1=-float(NPADROWS), scalar2=None, op0=OP.mult)
                    nc.vector.tensor_add(out=cor[:], in0=cor[:], in1=arro[:, 0:2])
                    mean_t = stp.tile([1, 1], f32)
                    nc.vector.tensor_scalar(out=mean_t[:], in0=cor[:, 0:1],
                                            scalar1=1.0 / CNT, scalar2=None, op0=OP.mult)
                    ex2 = stp.tile([1, 1], f32)
                    nc.vector.tensor_scalar(out=ex2[:], in0=cor[:, 1:2],
                                            scalar1=1.0 / CNT, scalar2=None, op0=OP.mult)
                    m2 = stp.tile([1, 1], f32)
                    nc.vector.tensor_tensor(out=m2[:], in0=mean_t[:], in1=mean_t[:], op=OP.mult)
                    var = stp.tile([1, 1], f32)
                    nc.vector.tensor_tensor(out=var[:], in0=ex2[:], in1=m2[:], op=OP.subtract)
                    nc.scalar.sqrt(var[:], var[:])
                    nc.vector.tensor_scalar(out=var[:], in0=var[:], scalar1=EPS,
                                            scalar2=None, op0=OP.add)
                    rstd = stp.tile([1, 1], f32)
                    nc.vector.reciprocal(rstd[:], var[:])
                    pack = stp.tile([1, 2], f32)
                    nc.vector.tensor_copy(out=pack[:, 0:1], in_=mean_t[:])
                    nc.vector.tensor_copy(out=pack[:, 1:2], in_=rstd[:])
                    mr = bcast_col(pack, stp)
                    a_bc = stp.tile([P, HID], f32)
                    c_bc = stp.tile([P, HID], f32)
                    nc.vector.tensor_scalar(out=a_bc[:], in0=wsb["ln2w_bc"][:],
                                            scalar1=mr[:, 1:2], scalar2=None, op0=OP.mult)
                    nc.vector.tensor_scalar(out=c_bc[:], in0=a_bc[:],
                                            scalar1=mr[:, 0:1], scalar2=None, op0=OP.mult)
                    nc.vector.tensor_tensor(out=c_bc[:], in0=wsb["ln2b_bc"][:],
                                            in1=c_bc[:], op=OP.subtract)
                    _ab = a_bc[:]
                    a_bc3 = bass.AP(_ab.tensor, _ab.offset,
                                    [list(_ab.ap[0]), [0, NT], list(_ab.ap[1])])
                    nc.vector.tensor_tensor(out=accum2[:], in0=accum2[:],
                                            in1=a_bc3, op=OP.mult)
                    _cb = c_bc[:]
                    c_bc3 = bass.AP(_cb.tensor, _cb.offset,
                                    [list(_cb.ap[0]), [0, NT], list(_cb.ap[1])])
                    nc.vector.tensor_tensor(out=accum2[:], in0=accum2[:],
                                            in1=c_bc3, op=OP.add)
                    nc.scalar.activation(out=accum2[:].rearrange("p a b -> p (a b)"),
                                         in_=accum2[:].rearrange("p a b -> p (a b)"),
                                         func=AF.Prelu, alpha=a2)

                # =========================== pooling ===========================
                with tc.tile_pool(name="poolp", bufs=1) as plp, \
                     tc.tile_pool(name="pps", bufs=1, space="PSUM") as pps, \
                     nc.named_scope("tail"):
                    batch_sb = plp.tile([P, NT], f32)
                    nc.sync.dma_start(out=batch_sb[:], in_=batch_in[:])
                    psg = pps.tile([P, HID], f32, space="PSUM", tag="psg")
                    with tc.tile_pool(name="selg", bufs=4) as slg:
                        for t in range(NT):
                            selg = slg.tile([P, P], f32, tag="selg")
                            nc.vector.tensor_scalar(
                                out=selg[:], in0=iota_f[:],
                                scalar1=batch_sb[:, t:t + 1], scalar2=None,
                                op0=OP.is_equal)
                            nc.tensor.matmul(out=psg[:], lhsT=selg[:],
                                             rhs=accum2[:, t, :],
                                             start=(t == 0), stop=(t == NT - 1))
                    partial = plp.tile([P, HID], f32)
                    nc.vector.tensor_copy(out=partial[:], in_=psg[:])
                    # place rows at graph_base via one-hot matmuls; zero the rest
                    gb_sb = plp.tile([1, 1], f32)
                    nc.sync.dma_start(out=gb_sb[:], in_=gbase_in[:])
                    gb_col = bcast_col(gb_sb, plp)           # [128,1]
                    loc_col = plp.tile([P, 1], f32)
                    nc.vector.tensor_copy(out=loc_col[:], in_=iota_f[:, 0:1])
                    # iota column: iota_f[:,0:1] is all zeros (values are along free dim)
                    # need per-partition index 0..127: use iota with channel_multiplier=1
                    pidx_i = plp.tile([P, 1], i32)
                    nc.gpsimd.iota(pidx_i[:], pattern=[[0, 1]], base=0, channel_multiplier=1)
                    pidx = plp.tile([P, 1], f32)
                    nc.vector.tensor_copy(out=pidx[:], in_=pidx_i[:])
                    nc.vector.tensor_add(out=loc_col[:], in0=pidx[:], in1=gb_col[:])
                    zero_t = plp.tile([P, HID], f32)
                    nc.vector.memset(zero_t[:], 0.0)
                    for j in range(GT + 1):
                        nc.sync.dma_start(out=pool_in[j * P:(j + 1) * P, :], in_=zero_t[:])
                    with tc.tile_pool(name="plc", bufs=2) as plc, \
                         tc.tile_pool(name="ppl", bufs=2, space="PSUM") as ppl:
                        for j in range(GT):
                            sh_col = plc.tile([P, 1], f32, tag="shc")
                            nc.vector.tensor_scalar(out=sh_col[:], in0=loc_col[:],
                                                    scalar1=-float(j * P), scalar2=None,
                                                    op0=OP.add)
                            selj = plc.tile([P, P], f32, tag="selj")
                            nc.vector.tensor_scalar(out=selj[:], in0=iota_f[:],
                                                    scalar1=sh_col[:], scalar2=None,
                                                    op0=OP.is_equal)
                            psj = ppl.tile([P, HID], f32, space="PSUM", tag="psj")
                            nc.tensor.matmul(out=psj[:], lhsT=selj[:], rhs=partial[:],
                                             start=True, stop=True)
                            oj = plc.tile([P, HID], f32, tag="oj")
                            nc.vector.tensor_copy(out=oj[:], in_=psj[:])
                            nc.sync.dma_start(out=pool_in[j * P:(j + 1) * P, :], in_=oj[:])
                    nc.gpsimd.collective_compute(
                        "AllReduce", OP.add, replica_groups=[CORE_IDS],
                        ins=[pool_in[:]], outs=[pool_out[:]])

                    # ---- head (redundant on every core) ----
                    cnt_sb = plp.tile([P, GT], f32)
                    nc.sync.dma_start(out=cnt_sb[:], in_=cnt_in[:])
                    nc.vector.tensor_scalar(out=cnt_sb[:], in0=cnt_sb[:], scalar1=1.0,
                                            scalar2=None, op0=OP.max)
                    rec_sb = plp.tile([P, GT], f32)
                    nc.vector.reciprocal(rec_sb[:], cnt_sb[:])
                    pooled = plp.tile([P, GT, HID], f32)
                    nc.sync.dma_start(
                        out=pooled[:],
                        in_=pool_out[0:G, :].rearrange("(a b) d -> b a d", b=P))
                    for j in range(GT):
                        nc.vector.tensor_scalar(out=pooled[:, j, :], in0=pooled[:, j, :],
                                                scalar1=rec_sb[:, j:j + 1], scalar2=None,
                                                op0=OP.mult)
                    # pooledT [128ch, NH, G]
                    pooledT = plp.tile([P, NH, G], f32)
                    with tc.tile_pool(name="trp", bufs=2) as trp:
                        for j in range(GT):
                            for h in range(NH):
                                ptp = psc.tile([P, P], f32, space="PSUM", tag="psc_scratch")
                                nc.tensor.transpose(
                                    out=ptp[:], in_=pooled[:, j, h * P:(h + 1) * P],
                                    identity=ident[:])
                                nc.vector.tensor_copy(
                                    out=pooledT[:, h, j * P:(j + 1) * P], in_=ptp[:])
                    # h1 = pooled @ Wl1 + bl1 : [G, 128]
                    HW = HID // 2
                    h1 = plp.tile([P, GT, HW], f32)
                    with tc.tile_pool(name="ph1", bufs=2, space="PSUM") as ph1:
                        for j in range(GT):
                            psh = ph1.tile([P, HW], f32, space="PSUM", tag="psh")
                            for h in range(NH):
                                nc.tensor.matmul(
                                    out=psh[:], lhsT=pooledT[:, h, j * P:(j + 1) * P],
                                    rhs=wsb["wl1_kt"][:, h, :], start=(h == 0), stop=(h == NH - 1))
                            nc.vector.tensor_add(out=h1[:, j, :], in0=psh[:],
                                                 in1=wsb["bl1_bc"][:])
                    # lnm (local, exact: G*HW elements)
                    s_col = plp.tile([P, 1], f32)
                    nc.vector.tensor_reduce(out=s_col[:], in_=h1[:].rearrange("p a b -> p (a b)"),
                                            axis=mybir.AxisListType.X, op=OP.add)
                    sq_col = plp.tile([P, 1], f32)
                    sqt2 = plp.tile([P, GT * HW], f32)
                    nc.scalar.activation(out=sqt2[:], in_=h1[:].rearrange("p a b -> p (a b)"),
                                         func=AF.Square, accum_out=sq_col[:])
                    both = plp.tile([P, 2], f32)
                    nc.vector.tensor_copy(out=both[:, 0:1], in_=s_col[:])
                    nc.vector.tensor_copy(out=both[:, 1:2], in_=sq_col[:])
                    tot = part_sum(both, plp)
                    CNTM = float(G * HW)
                    mean_t = plp.tile([1, 1], f32)
                    nc.vector.tensor_scalar(out=mean_t[:], in0=tot[:, 0:1],
                                            scalar1=1.0 / CNTM, scalar2=None, op0=OP.mult)
                    ex2 = plp.tile([1, 1], f32)
                    nc.vector.tensor_scalar(out=ex2[:], in0=tot[:, 1:2],
                                            scalar1=1.0 / CNTM, scalar2=None, op0=OP.mult)
                    m2 = plp.tile([1, 1], f32)
                    nc.vector.tensor_tensor(out=m2[:], in0=mean_t[:], in1=mean_t[:], op=OP.mult)
                    var = plp.tile([1, 1], f32)
                    nc.vector.tensor_tensor(out=var[:], in0=ex2[:], in1=m2[:], op=OP.subtract)
                    nc.scalar.sqrt(var[:], var[:])
                    nc.vector.tensor_scalar(out=var[:], in0=var[:], scalar1=EPS,
                                            scalar2=None, op0=OP.add)
                    rstd = plp.tile([1, 1], f32)
                    nc.vector.reciprocal(rstd[:], var[:])
                    pack = plp.tile([1, 2], f32)
                    nc.vector.tensor_copy(out=pack[:, 0:1], in_=mean_t[:])
                    nc.vector.tensor_copy(out=pack[:, 1:2], in_=rstd[:])
                    mr = bcast_col(pack, plp)
                    a_bc = plp.tile([P, HW], f32)
                    c_bc = plp.tile([P, HW], f32)
                    nc.vector.tensor_scalar(out=a_bc[:], in0=wsb["lnmw_bc"][:],
                                            scalar1=mr[:, 1:2], scalar2=None, op0=OP.mult)
                    nc.vector.tensor_scalar(out=c_bc[:], in0=a_bc[:],
                                            scalar1=mr[:, 0:1], scalar2=None, op0=OP.mult)
                    nc.vector.tensor_tensor(out=c_bc[:], in0=wsb["lnmb_bc"][:],
                                            in1=c_bc[:], op=OP.subtract)
                    for j in range(GT):
                        nc.vector.tensor_tensor(out=h1[:, j, :], in0=h1[:, j, :],
                                                in1=a_bc[:], op=OP.mult)
                        nc.vector.tensor_add(out=h1[:, j, :], in0=h1[:, j, :], in1=c_bc[:])
                    nc.scalar.activation(out=h1[:].rearrange("p a b -> p (a b)"),
                                         in_=h1[:].rearrange("p a b -> p (a b)"),
                                         func=AF.Prelu, alpha=am)
                    # out = h1' @ wl2 + bl2
                    outt = plp.tile([P, GT, OUT], f32)
                    with tc.tile_pool(name="of", bufs=2) as ofp:
                        for j in range(GT):
                            ptp = psc.tile([P, P], f32, space="PSUM", tag="psc_scratch")
                            nc.tensor.transpose(out=ptp[:], in_=h1[:, j, :],
                                                identity=ident[:])
                            h1t = ofp.tile([P, P], f32, tag="h1t")
                            nc.vector.tensor_copy(out=h1t[:], in_=ptp[:])
                            pso = psc.tile([P, OUT], f32, space="PSUM", tag="psc_scratch")
                            nc.tensor.matmul(out=pso[:], lhsT=h1t[:], rhs=wsb["wl2"][:],
                                             start=True, stop=True)
                            nc.vector.tensor_add(out=outt[:, j, :], in0=pso[:],
                                                 in1=wsb["bl2_bc"][:, 0:OUT])
                    nc.sync.dma_start(
                        out=out_ext[:].rearrange("(a b) d -> b a d", b=P),
                        in_=outt[:])

    nc.compile()
    return nc


# ----------------------------------------------------------------- entry point

def _run(cfg, inputs, use_sim=False):
    import sys
    if '/opt/trn_rl_repo' not in sys.path:
        sys.path.insert(0, '/opt/trn_rl_repo')
    pos = np.asarray(inputs["pos"], np.float32)
    ei = np.asarray(inputs["edge_index"], np.int64)
    batch = np.asarray(inputs["batch"], np.int64)
    meta, core_ins = host_prep(cfg, pos, ei, batch)
    w = _prep_weights(cfg, inputs)
    nc = build_program(cfg, meta, w)
    wnames = ["w1", "b1_cols", "ln1w_cols", "ln1b_cols", "w2_kt", "b2_bc",
              "ln2w_bc", "ln2b_bc", "wl1_kt", "bl1_bc", "lnmw_bc", "lnmb_bc",
              "wl2", "bl2_bc"]
    for ci in range(cfg["NCORES"]):
        for k in wnames:
            core_ins[ci][k] = np.asarray(w[k], np.float32)
    if use_sim:
        from concourse.bass_interp import MultiCoreSim
        sim = MultiCoreSim(nc, cfg["NCORES"])
        for ci in range(cfg["NCORES"]):
            for k, v in core_ins[ci].items():
                sim.cores[ci].tensor(k)[:] = v
        sim.simulate()
        return np.array(sim.cores[0].tensor("out")), None
    from concourse.bass_utils import run_bass_kernel_spmd
    res = run_bass_kernel_spmd(nc, core_ins, list(range(cfg["NCORES"])))
    return res.results[0]["out"], res


def kernel(**inputs):
    out, _ = _run(_cfg_full(), inputs)
    return out



# revision 21
# speedup vs baseline: 2.2347x; 2.2347x over previous
"""GCN (2x GCNConv + graph-layernorm + prelu + mean-pool + MLP head) on 8 trn2 cores.

v2 strategy (dst-sharded graph parallel):
  - nodes and their incoming edges sharded 8 ways by dst; weights replicated.
  - L1: host materializes per-destination padded source lists (pure integer
    indexing / re-layout of the input tensors): pos[src] and deg[src] streams
    laid out [128, NT, 2, K] / [128, NT, K] bf16. Device computes
    rsqrt(deg[src]), multiplies and does one strided reduction -> agg1.
    This removes all per-edge DMA-gather descriptors for layer 1.
  - x1 = prelu(LN(agg1*dinv @ W1 + b1)); LN stats via AllReduce.
  - h2 = x1 @ W2 (bf16 matmuls), scaled by dinv -> fp16 table rows; the table
    row order is quarter-major-permuted so 4 pipelined AllGathers produce
    chunk-contiguous regions addressable with int16 gather indices.
  - L2: per-edge dma_gather of fp16 table rows (optionally two 256B half-row
    gathers sharing one index tile), one-hot scatter matmuls PSUM-chained per
    dst tile across all 4 chunks (supertile-major), y2 kept in bf16.
  - LN2 stats AllReduce; mean-pool via one-hot matmuls; pooled AllReduce;
    MLP head computed redundantly on every core.
All floating point compute happens on device; the host only shards/sorts/pads
integer index metadata and re-lays-out input tensors.
"""

import numpy as np
import ml_dtypes

P = 128
BF16 = ml_dtypes.bfloat16


def _cfg_tiny():
    return dict(
        N=1900, E0=8000, G=128, IN_C=2, HID=256, OUT=16,
        NCORES=8, SH=1024, CT=2, ST=2, HALF_SPLIT=False,
    )


def _cfg_full():
    return dict(
        N=100000, E0=3200000, G=512, IN_C=2, HID=256, OUT=16,
        NCORES=8, SH=12544, CT=16, ST=8, HALF_SPLIT=False,
    )


# ----------------------------------------------------------------- host prep

def _wrap_idx(ix):
    """dma_gather idx layout: idx i -> [i%16 + 16k, i//16] for all k (replicated)."""
    m = ix.reshape(-1, 16).T
    return np.tile(m, (8, 1)).astype(np.int16)


def host_prep(cfg, pos, edge_index, batch):
    c = cfg
    N, E0, G, SH, CT, ST = c["N"], c["E0"], c["G"], c["SH"], c["CT"], c["ST"]
    NC = c["NCORES"]
    NT = SH // P
    NPAD = SH * NC
    NST = (NT + ST - 1) // ST

    # quarter/chunk layout of the h-table (table rows permuted quarter-major)
    base_qt, rem = divmod(NT, 4)
    qtiles = [base_qt + (1 if i < rem else 0) for i in range(4)]
    qstart_t = np.concatenate([[0], np.cumsum(qtiles)])      # tile boundaries
    QH = [qt * P for qt in qtiles]                           # rows per core-quarter
    B = np.concatenate([[0], np.cumsum([NC * q for q in QH])]).astype(np.int64)
    assert B[-1] == NPAD
    assert max(NC * q for q in QH) <= 32768, "chunk exceeds int16 idx range"

    src = np.concatenate([edge_index[0], np.arange(N, dtype=np.int64)])
    dst = np.concatenate([edge_index[1], np.arange(N, dtype=np.int64)])
    E = src.shape[0]
    deg = np.bincount(dst, minlength=NPAD).astype(np.int64)  # in-degree w/ loops

    # ---- L1 per-destination source lists -------------------------------
    K = int(deg.max())
    K = (K + 7) // 8 * 8
    so = np.argsort(dst, kind="stable")
    dsts = dst[so]
    srcs = src[so]
    starts = np.searchsorted(dsts, np.arange(NPAD))
    slot = np.arange(E) - starts[dsts]
    srcmat = np.full((NPAD, K), N, np.int64)                 # sentinel N
    srcmat[dsts, slot] = srcs
    posp = np.vstack([np.asarray(pos, np.float32), np.zeros((1, 2), np.float32)])
    degp = np.ones(N + 1, np.float32)
    degp[:N] = np.maximum(deg[:N], 1)

    # ---- table row permutation ----------------------------------------
    def table_row(n):
        ci = n // SH
        r = n % SH
        t = r // P
        q = np.searchsorted(qstart_t, t, side="right") - 1
        return B[q] + ci * np.asarray(QH)[q] + (r - qstart_t[q] * P)

    trow_all = table_row(srcs)

    # ---- L2 per-core edge cells ---------------------------------------
    bounds = np.searchsorted(dsts, np.arange(NC + 1) * SH)
    NCELL = 4 * NT
    counts = np.zeros((NC, 4, NT), np.int64)
    percore = []
    for ci in range(NC):
        sl = slice(bounds[ci], bounds[ci + 1])
        tr = trow_all[sl]
        dl = dsts[sl] - ci * SH
        tt = dl // P
        qq = np.searchsorted(B, tr, side="right") - 1
        o = np.lexsort((dl, tt, qq))
        tr, dl, tt, qq = tr[o], dl[o], tt[o], qq[o]
        counts[ci] = np.bincount(qq * NT + tt, minlength=NCELL).reshape(4, NT)
        percore.append((tr, dl, tt, qq))

    tiles_cell = (counts.max(axis=0) + P - 1) // P           # [4, NT]
    assert (tiles_cell.sum(axis=0) > 0).all(), "dst tile with no edges"
    TOT = int(tiles_cell.sum())

    # stream cell order: chunk-major (q, t); per-cell first/last tile flags
    tile_meta = []                        # (t, first, last) per stream tile
    CALLS = []                            # (q, ntiles)
    cell_order = []
    for q in range(4):
        k_q = 0
        for t in range(NT):
            n_tiles = int(tiles_cell[q, t])
            for j in range(n_tiles):
                tile_meta.append((t, j == 0, j == n_tiles - 1))
            k_q += n_tiles
            if n_tiles:
                cell_order.append((q, t))
        while k_q > 0:
            k = min(CT, k_q)
            CALLS.append((q, k))
            k_q -= k
    assert len(tile_meta) == TOT

    # per-core idx / dst column streams in stream order
    idx_wrapped, dst_cols = [], []
    for ci in range(NC):
        tr, dl, tt, qq = percore[ci]
        cnt = counts[ci]
        # edge offsets per cell in (q, t) sort order == (s, q, t) stream order?
        # stream order is (s asc, q asc, t asc); sorted edge order is
        # (s, q, t, dl). build per-cell start offsets from the sorted arrays.
        keys = qq * NT + tt
        cell_sizes = np.bincount(keys, minlength=NCELL)
        cell_off = np.concatenate([[0], np.cumsum(cell_sizes)])[:-1]

        idxs = np.zeros(TOT * P, np.int16)
        dcol = np.full(TOT * P, 999.0, np.float16)
        soff = 0
        for (q, t) in cell_order:
            n = int(cnt[q, t])
            e0 = int(cell_off[q * NT + t])
            slots = int(tiles_cell[q, t]) * P
            idxs[soff:soff + n] = (tr[e0:e0 + n] - B[q]).astype(np.int16)
            dcol[soff:soff + n] = (dl[e0:e0 + n] - t * P).astype(np.float16)
            soff += slots
        assert soff == TOT * P

        # wrap idx per call
        parts, off2 = [], 0
        for (_q, k) in CALLS:
            parts.append(_wrap_idx(idxs[off2:off2 + k * P]))
            off2 += k * P
        idx_wrapped.append(np.concatenate(parts, axis=1))    # [128, TOT*8]
        dst_cols.append(dcol.reshape(TOT, P).T.copy())       # [128, TOT]

    # ---- pooling metadata ---------------------------------------------
    gbase = np.zeros(NC, np.int32)
    batch_local = np.full((NC, SH), 999.0, np.float32)
    for ci in range(NC):
        lo, hi = ci * SH, min((ci + 1) * SH, N)
        if lo < N:
            gbase[ci] = batch[lo]
            batch_local[ci, :hi - lo] = (batch[lo:hi] - batch[lo]).astype(np.float32)
            assert batch[hi - 1] - batch[lo] < P - 2, "too many graphs in one shard"
    cnts = np.bincount(batch, minlength=G).astype(np.float32)
    GT = (G + P - 1) // P
    cnt_dev = np.zeros((P, GT), np.float32)
    for g in range(G):
        cnt_dev[g % P, g // P] = cnts[g]

    meta = dict(
        NPAD=NPAD, NT=NT, TOT=TOT, K=K, NST=NST,
        tiles_cell=tiles_cell, CALLS=CALLS, tile_meta=tile_meta,
        qtiles=qtiles, qstart_t=qstart_t, QH=QH, B=B,
        NPADROWS=NPAD - N,
    )

    ins = []
    for ci in range(NC):
        lo, hi = ci * SH, (ci + 1) * SH
        sm = srcmat[lo:hi]                                   # [SH, K]
        pdev = posp[sm].reshape(NT, P, K, 2).transpose(1, 0, 3, 2)
        ddev = degp[sm].reshape(NT, P, K).transpose(1, 0, 2)
        degsh = deg[lo:hi].astype(np.float32).reshape(NT, P).T.copy()
        ins.append(dict(
            pos_st=np.ascontiguousarray(pdev).astype(BF16),
            deg_st=np.ascontiguousarray(ddev).astype(BF16),
            deg_shard=degsh,
            idxs=idx_wrapped[ci], dstc=dst_cols[ci],
            batchl=batch_local[ci].reshape(NT, P).T.astype(np.float32),
            cntg=cnt_dev,
            gbase=np.array([[float(gbase[ci])]], np.float32),
        ))
    return meta, ins


def _prep_weights(cfg, W):
    """Re-layout weights for device (pure replication / transpose-free reshapes)."""
    c = cfg
    HID, OUT, IN_C, G = c["HID"], c["OUT"], c["IN_C"], c["G"]
    NH = HID // P
    w = {}
    w["w1"] = W["w_conv1"].astype(np.float32)                        # [2, 256]
    w["b1_cols"] = np.asarray(W["b_conv1"]).reshape(NH, P).T.copy()
    w["ln1w_cols"] = np.asarray(W["ln1_w"]).reshape(NH, P).T.copy()
    w["ln1b_cols"] = np.asarray(W["ln1_b"]).reshape(NH, P).T.copy()
    w["w2_kt"] = np.ascontiguousarray(
        np.asarray(W["w_conv2"]).reshape(NH, P, HID).transpose(1, 0, 2))
    w["b2_bc"] = np.tile(np.asarray(W["b_conv2"])[None, :], (P, 1)).astype(np.float32)
    w["ln2w_bc"] = np.tile(np.asarray(W["ln2_w"])[None, :], (P, 1)).astype(np.float32)
    w["ln2b_bc"] = np.tile(np.asarray(W["ln2_b"])[None, :], (P, 1)).astype(np.float32)
    w["wl1_kt"] = np.ascontiguousarray(
        np.asarray(W["w_lin1"]).reshape(NH, P, HID // 2).transpose(1, 0, 2))
    w["bl1_bc"] = np.tile(np.asarray(W["b_lin1"])[None, :], (P, 1)).astype(np.float32)
    w["lnmw_bc"] = np.tile(np.asarray(W["lnm_w"])[None, :], (P, 1)).astype(np.float32)
    w["lnmb_bc"] = np.tile(np.asarray(W["lnm_b"])[None, :], (P, 1)).astype(np.float32)
    w["wl2"] = np.asarray(W["w_lin2"]).astype(np.float32)            # [128, 16]
    w["bl2_bc"] = np.tile(np.asarray(W["b_lin2"])[None, :], (P, 1)).astype(np.float32)
    w["a1"] = float(W["a1"]); w["a2"] = float(W["a2"]); w["am"] = float(W["am"])
    return w


# ----------------------------------------------------------------- device build

def build_program(cfg, meta, weights):
    import concourse.bass as bass
    import concourse.mybir as mybir
    import concourse.tile as tile
    from concourse import bacc
    from concourse.masks import make_identity

    c = cfg
    dt = mybir.dt
    N, G, HID, OUT, IN_C = c["N"], c["G"], c["HID"], c["OUT"], c["IN_C"]
    SH, CT, ST = c["SH"], c["CT"], c["ST"]
    NCORES = c["NCORES"]
    HALF = c["HALF_SPLIT"]
    NPAD, NT, TOT, K, NST = meta["NPAD"], meta["NT"], meta["TOT"], meta["K"], meta["NST"]
    CALLS, tile_meta = meta["CALLS"], meta["tile_meta"]
    tiles_cell = meta["tiles_cell"]
    qtiles, qstart_t, QH, B = meta["qtiles"], meta["qstart_t"], meta["QH"], meta["B"]
    NH = HID // P
    GT = (G + P - 1) // P
    NPADROWS = meta["NPADROWS"]
    EPS = 1e-5
    CORE_IDS = list(range(NCORES))
    f32, f16, bf16, i16, i32 = dt.float32, dt.float16, dt.bfloat16, dt.int16, dt.int32
    AF = mybir.ActivationFunctionType
    OP = mybir.AluOpType

    nc = bacc.Bacc("TRN2", debug=False, num_devices=NCORES, num_swdge_queues=4)

    # ---- I/O ----
    pos_st_in = nc.declare_dram_parameter("pos_st", [P, NT, IN_C, K], bf16, isOutput=False)
    deg_st_in = nc.declare_dram_parameter("deg_st", [P, NT, K], bf16, isOutput=False)
    degs_in = nc.declare_dram_parameter("deg_shard", [P, NT], f32, isOutput=False)
    idx_in = nc.declare_dram_parameter("idxs", [P, TOT * 8], i16, isOutput=False)
    dstc_in = nc.declare_dram_parameter("dstc", [P, TOT], f16, isOutput=False)
    batch_in = nc.declare_dram_parameter("batchl", [P, NT], f32, isOutput=False)
    cnt_in = nc.declare_dram_parameter("cntg", [P, GT], f32, isOutput=False)
    gbase_in = nc.declare_dram_parameter("gbase", [1, 1], f32, isOutput=False)
    wt = {}
    wspec = dict(
        w1=[IN_C, HID], b1_cols=[P, NH], ln1w_cols=[P, NH], ln1b_cols=[P, NH],
        w2_kt=[P, NH, HID], b2_bc=[P, HID], ln2w_bc=[P, HID], ln2b_bc=[P, HID],
        wl1_kt=[P, NH, HID // 2], bl1_bc=[P, HID // 2], lnmw_bc=[P, HID // 2],
        lnmb_bc=[P, HID // 2], wl2=[HID // 2, OUT], bl2_bc=[P, OUT],
    )
    for k_, shp in wspec.items():
        wt[k_] = nc.declare_dram_parameter(k_, shp, f32, isOutput=False)
    out_ext = nc.declare_dram_parameter("out", [G, OUT], f32, isOutput=True)
    DBG = c.get("DBG", False)
    if DBG:
        dbg_agg = nc.declare_dram_parameter("dbg_agg", [P, NT * IN_C], f32, isOutput=True)
        dbg_x1 = nc.declare_dram_parameter("dbg_x1", [P, NH * 256], f32, isOutput=True)
        dbg_y1 = nc.declare_dram_parameter("dbg_y1", [P, NH * 256], f32, isOutput=True)
        dbg_st = nc.declare_dram_parameter("dbg_st", [1, 16], f32, isOutput=True)
        dbg_y2 = nc.declare_dram_parameter("dbg_y2", [P, 4 * HID], f32, isOutput=True)

    # ---- internal DRAM ----
    gshard_q = [nc.dram_tensor(f"gshard{q}", [QH[q], HID], f16) for q in range(4)]
    gtab_q = [nc.dram_tensor(f"gtab{q}", [NCORES * QH[q], HID], f16,
                             addr_space="Shared") for q in range(4)]
    st1_in = nc.dram_tensor("st1_in", [1, P], f32)
    st1_out = nc.dram_tensor("st1_out", [1, P], f32, addr_space="Shared")
    st2_in = nc.dram_tensor("st2_in", [1, P], f32)
    st2_out = nc.dram_tensor("st2_out", [1, P], f32, addr_space="Shared")
    POOLR = (GT + 1) * P
    pool_in = nc.dram_tensor("pool_in", [POOLR, HID], f32)
    pool_out = nc.dram_tensor("pool_out", [POOLR, HID], f32, addr_space="Shared")

    a1, a2, am = weights["a1"], weights["a2"], weights["am"]

    with tile.TileContext(nc) as tc:
        with tc.tile_pool(name="persist", bufs=1) as pp, \
             tc.tile_pool(name="psc", bufs=2, space="PSUM") as psc:
            # ---- persistent small tiles ----
            iota_i = pp.tile([P, P], i32)
            nc.gpsimd.iota(iota_i[:], pattern=[[1, P]], base=0, channel_multiplier=0)
            iota_h = pp.tile([P, P], f16)
            nc.vector.tensor_copy(out=iota_h[:], in_=iota_i[:])
            iota_f = pp.tile([P, P], f32)
            nc.vector.tensor_copy(out=iota_f[:], in_=iota_i[:])
            ident = pp.tile([P, P], f32)
            make_identity(nc, ident[:])
            ones_col = pp.tile([P, 1], f32)
            nc.vector.memset(ones_col[:], 1.0)
            ones_row = pp.tile([1, P], f32)
            nc.vector.memset(ones_row[:], 1.0)

            dstc16 = pp.tile([P, TOT], f16)
            nc.sync.dma_start(out=dstc16[:], in_=dstc_in[:])

            wsb = {}
            for k_, shp in wspec.items():
                wsb[k_] = pp.tile(shp, f32, name=f"w_{k_}")
                nc.sync.dma_start(out=wsb[k_][:], in_=wt[k_][:])
            w2b = pp.tile([P, NH, HID], bf16)
            nc.vector.tensor_copy(out=w2b[:], in_=wsb["w2_kt"][:])
            w1b = pp.tile([IN_C, HID], bf16)
            nc.vector.tensor_copy(out=w1b[:], in_=wsb["w1"][:])

            # dinv for this core's dst rows
            deg_s = pp.tile([P, NT], f32)
            nc.sync.dma_start(out=deg_s[:], in_=degs_in[:])
            nc.vector.tensor_scalar(out=deg_s[:], in0=deg_s[:], scalar1=1.0,
                                    scalar2=None, op0=OP.max)
            nc.scalar.sqrt(deg_s[:], deg_s[:])
            dinv_s = pp.tile([P, NT], f32)
            nc.vector.reciprocal(dinv_s[:], deg_s[:])

            # helper: cross-partition sum -> [1,k] sbuf tile
            def part_sum(src_col, w_):
                ps = psc.tile([1, src_col.shape[1]], f32, space="PSUM", tag="psc_s")
                nc.tensor.matmul(out=ps[:], lhsT=ones_col[:], rhs=src_col[:],
                                 start=True, stop=True)
                dstt = w_.tile([1, src_col.shape[1]], f32, tag="psum_scalar")
                nc.vector.tensor_copy(out=dstt[:], in_=ps[:])
                return dstt

            def bcast_col(vals_row, w_):
                k_ = vals_row.shape[1]
                ps = psc.tile([P, k_], f32, space="PSUM", tag="psc_s")
                nc.tensor.matmul(out=ps[:], lhsT=ones_row[:], rhs=vals_row[:],
                                 start=True, stop=True)
                o = w_.tile([P, k_], f32, tag="bcast_col")
                nc.vector.tensor_copy(out=o[:], in_=ps[:])
                return o

            def ln_stats_to_affine(tot_corr, cnt_total, lnw, lnb, stp, wide):
                """tot_corr [1,2] (sum, sumsq) -> affine (a,c) tiles [P, wide]."""
                mean_t = stp.tile([1, 1], f32, tag="mean")
                nc.vector.tensor_scalar(out=mean_t[:], in0=tot_corr[:, 0:1],
                                        scalar1=1.0 / cnt_total, scalar2=None, op0=OP.mult)
                ex2 = stp.tile([1, 1], f32, tag="ex2")
                nc.vector.tensor_scalar(out=ex2[:], in0=tot_corr[:, 1:2],
                                        scalar1=1.0 / cnt_total, scalar2=None, op0=OP.mult)
                m2 = stp.tile([1, 1], f32, tag="m2")
                nc.vector.tensor_tensor(out=m2[:], in0=mean_t[:], in1=mean_t[:], op=OP.mult)
                var = stp.tile([1, 1], f32, tag="var")
                nc.vector.tensor_tensor(out=var[:], in0=ex2[:], in1=m2[:], op=OP.subtract)
                nc.scalar.sqrt(var[:], var[:])
                nc.vector.tensor_scalar(out=var[:], in0=var[:], scalar1=EPS,
                                        scalar2=None, op0=OP.add)
                rstd = stp.tile([1, 1], f32, tag="rstd")
                nc.vector.reciprocal(rstd[:], var[:])
                pack = stp.tile([1, 2], f32, tag="pack")
                nc.vector.tensor_copy(out=pack[:, 0:1], in_=mean_t[:])
                nc.vector.tensor_copy(out=pack[:, 1:2], in_=rstd[:])
                mr = bcast_col(pack, stp)
                a_t = stp.tile([P, wide], f32, tag="a_t")
                c_t = stp.tile([P, wide], f32, tag="c_t")
                nc.vector.tensor_scalar(out=a_t[:], in0=lnw[:],
                                        scalar1=mr[:, 1:2], scalar2=None, op0=OP.mult)
                nc.vector.tensor_scalar(out=c_t[:], in0=a_t[:],
                                        scalar1=mr[:, 0:1], scalar2=None, op0=OP.mult)
                nc.vector.tensor_tensor(out=c_t[:], in0=lnb[:], in1=c_t[:], op=OP.subtract)
                return a_t, c_t


            def prelu_blocks(out_flat, in_flat, alpha, scr_pool, scr_dt):
                cols = in_flat.shape[1]
                BLKP = 4096
                for b0 in range(0, cols, BLKP):
                    b1 = min(b0 + BLKP, cols)
                    scr = scr_pool.tile([P, BLKP], scr_dt, tag="prelu_scr")
                    nc.vector.tensor_scalar(
                        out=scr[:, 0:b1 - b0], in0=in_flat[:, b0:b1],
                        scalar1=0.0, scalar2=float(alpha) - 1.0,
                        op0=OP.min, op1=OP.mult)
                    nc.vector.tensor_tensor(
                        out=out_flat[:, b0:b1], in0=in_flat[:, b0:b1],
                        in1=scr[:, 0:b1 - b0], op=OP.add)

            # =============================== L1 ===============================
            with tc.tile_pool(name="agg1p", bufs=1) as a1p:
                agg1 = a1p.tile([P, NT, IN_C], f32)
                with tc.tile_pool(name="l1s", bufs=1) as l1p, nc.named_scope("L1"):
                    ps_st = l1p.tile([P, NT, IN_C, K], bf16)
                    nc.sync.dma_start(out=ps_st[:], in_=pos_st_in[:])
                    dg_st = l1p.tile([P, NT, K], bf16)
                    nc.sync.dma_start(out=dg_st[:], in_=deg_st_in[:])
                    dsr = l1p.tile([P, NT, K], bf16)
                    with nc.allow_low_precision(reason="bf16 rsqrt of integer degs"):
                        nc.scalar.sqrt(dsr[:].rearrange("p a b -> p (a b)"),
                                       dg_st[:].rearrange("p a b -> p (a b)"))
                        nc.vector.reciprocal(dsr[:].rearrange("p a b -> p (a b)"),
                                             dsr[:].rearrange("p a b -> p (a b)"))
                    _d = dsr[:]
                    d_bc = bass.AP(_d.tensor, _d.offset,
                                   [list(_d.ap[0]), list(_d.ap[1]), [0, IN_C],
                                    list(_d.ap[2])])
                    nc.vector.tensor_tensor(out=ps_st[:], in0=ps_st[:], in1=d_bc,
                                            op=OP.mult)
                    nc.vector.tensor_reduce(
                        out=agg1[:].rearrange("p a b -> p (a b)"),
                        in_=ps_st[:].rearrange("p a b c -> p (a b) c"),
                        axis=mybir.AxisListType.X, op=OP.add)

                if DBG:
                    nc.sync.dma_start(out=dbg_agg[:],
                                      in_=agg1[:].rearrange("p a b -> p (a b)"))
                # ---- x1t = W1 @ (dinv*agg).T + b1 ; layout [128ch, NH, SH] ----
                with tc.tile_pool(name="x1", bufs=1) as x1p, nc.named_scope("mid"):
                    x1t = x1p.tile([P, NH, SH], bf16)
                    p1t_all = x1p.tile([IN_C, SH], bf16)
                    with tc.tile_pool(name="w1w", bufs=4) as wk:
                        for t in range(NT):
                            sc = wk.tile([P, IN_C], f32, tag="sc")
                            nc.vector.tensor_scalar(
                                out=sc[:], in0=agg1[:, t, :],
                                scalar1=dinv_s[:, t:t + 1], scalar2=None, op0=OP.mult)
                            pt = psc.tile([IN_C, P], f32, space="PSUM", tag="psc_s")
                            nc.tensor.transpose(out=pt[:], in_=sc[:], identity=ident[:])
                            nc.vector.tensor_copy(out=p1t_all[:, t * P:(t + 1) * P],
                                                  in_=pt[:])
                        BLK = 512
                        psw_cm = tc.tile_pool(name="psw", bufs=2, space="PSUM")
                        psw = psw_cm.__enter__()
                        for h in range(NH):
                            for b0 in range(0, SH, BLK):
                                b1_ = min(b0 + BLK, SH)
                                psx = psw.tile([P, BLK], f32, space="PSUM", tag="psc_w")
                                nc.tensor.matmul(
                                    out=psx[:, 0:b1_ - b0], lhsT=w1b[:, h * P:(h + 1) * P],
                                    rhs=p1t_all[:, b0:b1_],
                                    start=True, stop=True)
                                with nc.allow_low_precision(reason="bf16 y1"):
                                    nc.vector.tensor_scalar(
                                        out=x1t[:, h, b0:b1_], in0=psx[:, 0:b1_ - b0],
                                        scalar1=wsb["b1_cols"][:, h:h + 1], scalar2=None,
                                        op0=OP.add)

                    psw_cm.__exit__(None, None, None)
                    if DBG:
                        with tc.tile_pool(name="dbg0", bufs=1) as dbp:
                            d0 = dbp.tile([P, NH, 256], f32)
                            nc.vector.tensor_copy(out=d0[:], in_=x1t[:, :, 0:256])
                            nc.sync.dma_start(out=dbg_y1[:],
                                              in_=d0[:].rearrange("p a b -> p (a b)"))
                    # ---- ln1 stats (global over x1) ----
                    with tc.tile_pool(name="stats", bufs=1) as stp:
                        s_col = stp.tile([P, 1], f32)
                        nc.vector.tensor_reduce(out=s_col[:],
                                                in_=x1t[:].rearrange("p a b -> p (a b)"),
                                                axis=mybir.AxisListType.X, op=OP.add)
                        CHK = 2048
                        nchk = (NH * SH + CHK - 1) // CHK
                        sq_cols = stp.tile([P, nchk], f32)
                        sq_scr = stp.tile([P, CHK], f32)
                        x1flat = x1t[:].rearrange("p a b -> p (a b)")
                        for ck in range(nchk):
                            lo, hi = ck * CHK, min((ck + 1) * CHK, NH * SH)
                            nc.scalar.activation(out=sq_scr[:, 0:hi - lo], in_=x1flat[:, lo:hi],
                                                 func=AF.Square, accum_out=sq_cols[:, ck:ck + 1])
                        sq_col = stp.tile([P, 1], f32)
                        nc.vector.tensor_reduce(out=sq_col[:], in_=sq_cols[:],
                                                axis=mybir.AxisListType.X, op=OP.add)
                        both = stp.tile([P, 2], f32)
                        nc.vector.tensor_copy(out=both[:, 0:1], in_=s_col[:])
                        nc.vector.tensor_copy(out=both[:, 1:2], in_=sq_col[:])
                        tot = part_sum(both, stp)            # [1,2] local (sum, sumsq)
                        # b1 pad-row corrections
                        b1s_c = stp.tile([P, 2], f32)
                        nc.vector.tensor_copy(out=b1s_c[:, 0:1], in_=wsb["b1_cols"][:, 0:1])
                        nc.scalar.square(b1s_c[:, 1:2], wsb["b1_cols"][:, 0:1])
                        for h in range(1, NH):
                            nc.vector.tensor_add(out=b1s_c[:, 0:1], in0=b1s_c[:, 0:1],
                                                 in1=wsb["b1_cols"][:, h:h + 1])
                            sqh = stp.tile([P, 1], f32, tag="sqh")
                            nc.scalar.square(sqh[:], wsb["b1_cols"][:, h:h + 1])
                            nc.vector.tensor_add(out=b1s_c[:, 1:2], in0=b1s_c[:, 1:2],
                                                 in1=sqh[:])
                        b1tot = part_sum(b1s_c, stp)
                        arr = stp.tile([1, P], f32)
                        nc.vector.memset(arr[:], 0.0)
                        nc.vector.tensor_copy(out=arr[:, 0:2], in_=tot[:])
                        nc.sync.dma_start(out=st1_in[:], in_=arr[:])
                        nc.gpsimd.collective_compute(
                            "AllReduce", OP.add, replica_groups=[CORE_IDS],
                            ins=[st1_in[:]], outs=[st1_out[:]])
                        arro = stp.tile([1, P], f32)
                        nc.sync.dma_start(out=arro[:], in_=st1_out[:])
                        cor = stp.tile([1, 2], f32)
                        nc.vector.tensor_scalar(out=cor[:], in0=b1tot[:],
                                                scalar1=-float(NPADROWS), scalar2=None,
                                                op0=OP.mult)
                        nc.vector.tensor_add(out=cor[:], in0=cor[:], in1=arro[:, 0:2])
                        if DBG:
                            dstt = stp.tile([1, 16], f32)
                            nc.vector.memset(dstt[:], 0.0)
                            nc.vector.tensor_copy(out=dstt[:, 0:2], in_=tot[:])
                            nc.vector.tensor_copy(out=dstt[:, 2:4], in_=arro[:, 0:2])
                            nc.vector.tensor_copy(out=dstt[:, 4:6], in_=cor[:])
                            nc.sync.dma_start(out=dbg_st[:], in_=dstt[:])
                        acol, ccol = ln_stats_to_affine(
                            cor, float(N * HID), wsb["ln1w_cols"], wsb["ln1b_cols"],
                            stp, NH)
                        with nc.allow_low_precision(reason="bf16 x1 affine"):
                            for h in range(NH):
                                nc.vector.tensor_scalar(
                                    out=x1t[:, h, :], in0=x1t[:, h, :],
                                    scalar1=acol[:, h:h + 1], scalar2=ccol[:, h:h + 1],
                                    op0=OP.mult, op1=OP.add)
                    x1b = x1t
                    with tc.tile_pool(name="pr1", bufs=2) as prp:
                        prelu_blocks(x1b[:].rearrange("p a b -> p (a b)"),
                                     x1t[:].rearrange("p a b -> p (a b)"), a1, prp, bf16)

                    if DBG:
                        with tc.tile_pool(name="dbg1", bufs=1) as dbp:
                            d1 = dbp.tile([P, NH, 256], f32)
                            nc.vector.tensor_copy(out=d1[:], in_=x1b[:, :, 0:256])
                            nc.sync.dma_start(out=dbg_x1[:],
                                              in_=d1[:].rearrange("p a b -> p (a b)"))
                    # ---- h2 per quarter; AllGather per quarter ----
                    with tc.tile_pool(name="h2w", bufs=3) as h2w, \
                         tc.tile_pool(name="ph2", bufs=2, space="PSUM") as ph2:
                        for q in range(4):
                            for t in range(int(qstart_t[q]), int(qstart_t[q + 1])):
                                ps2 = ph2.tile([P, HID], f32, space="PSUM", tag="ph2")
                                for h in range(NH):
                                    nc.tensor.matmul(
                                        out=ps2[:], lhsT=x1b[:, h, t * P:(t + 1) * P],
                                        rhs=w2b[:, h, :], start=(h == 0), stop=(h == NH - 1))
                                g16 = h2w.tile([P, HID], f16, tag="g16")
                                nc.vector.tensor_scalar(
                                    out=g16[:], in0=ps2[:],
                                    scalar1=dinv_s[:, t:t + 1], scalar2=None, op0=OP.mult)
                                tl_ = t - int(qstart_t[q])
                                nc.sync.dma_start(
                                    out=gshard_q[q][tl_ * P:(tl_ + 1) * P, :], in_=g16[:])
                            with nc.named_scope(f"AG{q}"):
                                nc.gpsimd.collective_compute(
                                    "AllGather", OP.bypass, replica_groups=[CORE_IDS],
                                    ins=[gshard_q[q][:]],
                                    outs=[gtab_q[q][:]])

            # =============================== L2 ===============================
            with tc.tile_pool(name="y2p", bufs=1) as y2p:
                y2sb = y2p.tile([P, NT, HID], bf16)
                nc.vector.memset(y2sb[:].rearrange("p a b -> p (a b)"), 0.0)
                qtiles_tot = [int(tiles_cell[q].sum()) for q in range(4)]
                QMAX = max(qtiles_tot)
                with tc.tile_pool(name="gbp", bufs=8) as gbp, \
                     tc.tile_pool(name="selp", bufs=6) as selp, \
                     tc.tile_pool(name="idxp", bufs=2) as idxp, \
                     tc.tile_pool(name="paccp", bufs=3, space="PSUM") as paccp, \
                     nc.named_scope("L2agg"):
                    stream_tile = 0
                    off_cols = 0
                    call_i = 0
                    pacc = None
                    cur_q = -1
                    idx_sb = None
                    q_off = 0
                    for (q, kk) in CALLS:
                        if q != cur_q:
                            cur_q = q
                            idx_sb = idxp.tile([P, QMAX * 8], i16, tag="idx")
                            nc.sync.dma_start(
                                out=idx_sb[:, 0:qtiles_tot[q] * 8],
                                in_=idx_in[:, off_cols:off_cols + qtiles_tot[q] * 8])
                            q_off = 0
                        idx_t = idx_sb
                        selc = selp.tile([P, CT, P], f16, tag="sel")
                        _i = iota_h[:]
                        _db = dstc16[:, stream_tile:stream_tile + kk]
                        iota_bc = bass.AP(_i.tensor, _i.offset,
                                          [list(_i.ap[0]), [0, kk], list(_i.ap[1])])
                        dst_bc = bass.AP(_db.tensor, _db.offset,
                                         [list(_db.ap[0]), list(_db.ap[1]), [0, P]])
                        nc.vector.tensor_tensor(out=selc[:, 0:kk, :], in0=iota_bc,
                                                in1=dst_bc, op=OP.is_equal)
                        gb = gbp.tile([P, CT, HID], f16, tag="g")
                        nc.gpsimd.dma_gather(
                            out_ap=gb[:, 0:kk, :],
                            idxs_ap=idx_t[:, q_off:q_off + kk * 8],
                            in_ap=gtab_q[q][:],
                            num_idxs=kk * P, num_idxs_reg=kk * P,
                            elem_size=HID, single_packet=False,
                            queue_num=call_i % 4)
                        call_i += 1
                        for j in range(kk):
                            t, first, last = tile_meta[stream_tile]
                            if first:
                                pacc = paccp.tile([P, HID], f32, space="PSUM",
                                                  tag="pacc", name="pacc")
                            nc.tensor.matmul(out=pacc[:], lhsT=selc[:, j, :],
                                             rhs=gb[:, j, :], start=first, stop=last)
                            if last:
                                with nc.allow_low_precision(reason="bf16 y2 accum"):
                                    nc.vector.tensor_add(out=y2sb[:, t, :],
                                                         in0=y2sb[:, t, :], in1=pacc[:])
                            stream_tile += 1
                        off_cols += kk * 8
                        q_off += kk * 8

                    # y2 = dinv * agg + b2 (bulk, in place)
                    _dv = dinv_s[:]
                    dinv_bc = bass.AP(_dv.tensor, _dv.offset,
                                      [list(_dv.ap[0]), list(_dv.ap[1]), [0, HID]])
                    with nc.allow_low_precision(reason="bf16 y2"):
                        nc.vector.tensor_tensor(out=y2sb[:], in0=y2sb[:],
                                                in1=dinv_bc, op=OP.mult)
                        _b2 = wsb["b2_bc"][:]
                        b2_bc3 = bass.AP(_b2.tensor, _b2.offset,
                                         [list(_b2.ap[0]), [0, NT], list(_b2.ap[1])])
                        nc.vector.tensor_tensor(out=y2sb[:], in0=y2sb[:],
                                                in1=b2_bc3, op=OP.add)

                if DBG:
                    with tc.tile_pool(name="dbg2", bufs=1) as dbp:
                        d2 = dbp.tile([P, 4, HID], f32)
                        nc.vector.tensor_copy(out=d2[:], in_=y2sb[:, 0:4, :])
                        nc.sync.dma_start(out=dbg_y2[:],
                                          in_=d2[:].rearrange("p a b -> p (a b)"))
                # ---- ln2 stats + affine + prelu ----
                tprep_cm = tc.tile_pool(name="tprep", bufs=1)
                tprep = tprep_cm.__enter__()
                with tc.tile_pool(name="stats2", bufs=1) as stp:
                    s_col = stp.tile([P, 1], f32)
                    nc.vector.tensor_reduce(out=s_col[:],
                                            in_=y2sb[:].rearrange("p a b -> p (a b)"),
                                            axis=mybir.AxisListType.X, op=OP.add)
                    CHK2 = 2048
                    nchk2 = (NT * HID + CHK2 - 1) // CHK2
                    sq_cols2 = stp.tile([P, nchk2], f32)
                    sq_scr2 = stp.tile([P, CHK2], f32)
                    y2flat = y2sb[:].rearrange("p a b -> p (a b)")
                    for ck in range(nchk2):
                        lo, hi = ck * CHK2, min((ck + 1) * CHK2, NT * HID)
                        nc.scalar.activation(out=sq_scr2[:, 0:hi - lo], in_=y2flat[:, lo:hi],
                                             func=AF.Square, accum_out=sq_cols2[:, ck:ck + 1])
                    sq_col = stp.tile([P, 1], f32)
                    nc.vector.tensor_reduce(out=sq_col[:], in_=sq_cols2[:],
                                            axis=mybir.AxisListType.X, op=OP.add)
                    both = stp.tile([P, 2], f32)
                    nc.vector.tensor_copy(out=both[:, 0:1], in_=s_col[:])
                    nc.vector.tensor_copy(out=both[:, 1:2], in_=sq_col[:])
                    tot = part_sum(both, stp)
                    b2p = stp.tile([1, 2], f32)
                    nc.vector.tensor_reduce(out=b2p[:, 0:1], in_=wsb["b2_bc"][0:1, :],
                                            axis=mybir.AxisListType.X, op=OP.add)
                    b2sq = stp.tile([1, HID], f32)
                    nc.scalar.square(b2sq[:], wsb["b2_bc"][0:1, :])
                    nc.vector.tensor_reduce(out=b2p[:, 1:2], in_=b2sq[:],
                                            axis=mybir.AxisListType.X, op=OP.add)
                    arr = stp.tile([1, P], f32)
                    nc.vector.memset(arr[:], 0.0)
                    nc.vector.tensor_copy(out=arr[:, 0:2], in_=tot[:])
                    nc.sync.dma_start(out=st2_in[:], in_=arr[:])
                    nc.gpsimd.collective_compute(
                        "AllReduce", OP.add, replica_groups=[CORE_IDS],
                        ins=[st2_in[:]], outs=[st2_out[:]])
                    arro = stp.tile([1, P], f32)
                    nc.sync.dma_start(out=arro[:], in_=st2_out[:])
                    # --- tail prep hidden under the AllReduce latency ---
                    batch_sb = tprep.tile([P, NT], f32)
                    nc.sync.dma_start(out=batch_sb[:], in_=batch_in[:])
                    selg_all = tprep.tile([P, NT, P], bf16)
                    for t_ in range(NT):
                        nc.vector.tensor_scalar(
                            out=selg_all[:, t_, :], in0=iota_f[:],
                            scalar1=batch_sb[:, t_:t_ + 1], scalar2=None,
                            op0=OP.is_equal)
                    cor = stp.tile([1, 2], f32)
                    nc.vector.tensor_scalar(out=cor[:], in0=b2p[:],
                                            scalar1=-float(NPADROWS), scalar2=None,
                                            op0=OP.mult)
                    nc.vector.tensor_add(out=cor[:], in0=cor[:], in1=arro[:, 0:2])
                    a_bc, c_bc = ln_stats_to_affine(
                        cor, float(N * HID), wsb["ln2w_bc"], wsb["ln2b_bc"], stp, HID)
                    _ab = a_bc[:]
                    a_bc3 = bass.AP(_ab.tensor, _ab.offset,
                                    [list(_ab.ap[0]), [0, NT], list(_ab.ap[1])])
                    nc.vector.tensor_tensor(out=y2sb[:], in0=y2sb[:], in1=a_bc3,
                                            op=OP.mult)
                    _cb = c_bc[:]
                    c_bc3 = bass.AP(_cb.tensor, _cb.offset,
                                    [list(_cb.ap[0]), [0, NT], list(_cb.ap[1])])
                    nc.vector.tensor_tensor(out=y2sb[:], in0=y2sb[:], in1=c_bc3,
                                            op=OP.add)
                    with tc.tile_pool(name="pr2", bufs=2) as prp:
                        prelu_blocks(y2sb[:].rearrange("p a b -> p (a b)"),
                                     y2sb[:].rearrange("p a b -> p (a b)"), a2, prp, bf16)

                # =========================== pooling ===========================
                with tc.tile_pool(name="poolp", bufs=1) as plp, \
                     tc.tile_pool(name="pps", bufs=1, space="PSUM") as pps, \
                     nc.named_scope("tail"):
                    psg = pps.tile([P, HID], f32, space="PSUM", tag="psg")
                    for t in range(NT):
                        nc.tensor.matmul(out=psg[:], lhsT=selg_all[:, t, :],
                                         rhs=y2sb[:, t, :],
                                         start=(t == 0), stop=(t == NT - 1))
                    partial = plp.tile([P, HID], f32)
                    nc.vector.tensor_copy(out=partial[:], in_=psg[:])
                    # place rows at graph_base via one-hot matmuls; zero the rest
                    gb_sb = plp.tile([1, 1], f32)
                    nc.sync.dma_start(out=gb_sb[:], in_=gbase_in[:])
                    gb_col = bcast_col(gb_sb, plp)
                    pidx_i = plp.tile([P, 1], i32)
                    nc.gpsimd.iota(pidx_i[:], pattern=[[0, 1]], base=0, channel_multiplier=1)
                    pidx = plp.tile([P, 1], f32)
                    nc.vector.tensor_copy(out=pidx[:], in_=pidx_i[:])
                    loc_col = plp.tile([P, 1], f32)
                    nc.vector.tensor_add(out=loc_col[:], in0=pidx[:], in1=gb_col[:])
                    zero_t = plp.tile([P, HID], f32)
                    nc.vector.memset(zero_t[:], 0.0)
                    for j in range(GT + 1):
                        nc.sync.dma_start(out=pool_in[j * P:(j + 1) * P, :], in_=zero_t[:])
                    with tc.tile_pool(name="plc", bufs=2) as plc, \
                         tc.tile_pool(name="ppl", bufs=2, space="PSUM") as ppl:
                        for j in range(GT):
                            sh_col = plc.tile([P, 1], f32, tag="shc")
                            nc.vector.tensor_scalar(out=sh_col[:], in0=loc_col[:],
                                                    scalar1=-float(j * P), scalar2=None,
                                                    op0=OP.add)
                            selj = plc.tile([P, P], f32, tag="selj")
                            nc.vector.tensor_scalar(out=selj[:], in0=iota_f[:],
                                                    scalar1=sh_col[:], scalar2=None,
                                                    op0=OP.is_equal)
                            psj = ppl.tile([P, HID], f32, space="PSUM", tag="psj")
                            nc.tensor.matmul(out=psj[:], lhsT=selj[:], rhs=partial[:],
                                             start=True, stop=True)
                            oj = plc.tile([P, HID], f32, tag="oj")
                            nc.vector.tensor_copy(out=oj[:], in_=psj[:])
                            nc.sync.dma_start(out=pool_in[j * P:(j + 1) * P, :], in_=oj[:])
                    nc.gpsimd.collective_compute(
                        "AllReduce", OP.add, replica_groups=[CORE_IDS],
                        ins=[pool_in[:]], outs=[pool_out[:]])

                    # ---- head (redundant on every core) ----
                    cnt_sb = plp.tile([P, GT], f32)
                    nc.sync.dma_start(out=cnt_sb[:], in_=cnt_in[:])
                    nc.vector.tensor_scalar(out=cnt_sb[:], in0=cnt_sb[:], scalar1=1.0,
                                            scalar2=None, op0=OP.max)
                    rec_sb = plp.tile([P, GT], f32)
                    nc.vector.reciprocal(rec_sb[:], cnt_sb[:])
                    pooled = plp.tile([P, GT, HID], f32)
                    nc.sync.dma_start(
                        out=pooled[:],
                        in_=pool_out[0:G, :].rearrange("(a b) d -> b a d", b=P))
                    for j in range(GT):
                        nc.vector.tensor_scalar(out=pooled[:, j, :], in0=pooled[:, j, :],
                                                scalar1=rec_sb[:, j:j + 1], scalar2=None,
                                                op0=OP.mult)
                    pooledT = plp.tile([P, NH, G], f32)
                    for j in range(GT):
                        for h in range(NH):
                            ptp = psc.tile([P, P], f32, space="PSUM", tag="psc_s")
                            nc.tensor.transpose(
                                out=ptp[:], in_=pooled[:, j, h * P:(h + 1) * P],
                                identity=ident[:])
                            nc.vector.tensor_copy(
                                out=pooledT[:, h, j * P:(j + 1) * P], in_=ptp[:])
                    HW = HID // 2
                    h1 = plp.tile([P, GT, HW], f32)
                    with tc.tile_pool(name="ph1", bufs=2, space="PSUM") as ph1:
                        for j in range(GT):
                            psh = ph1.tile([P, HW], f32, space="PSUM", tag="psh")
                            for h in range(NH):
                                nc.tensor.matmul(
                                    out=psh[:], lhsT=pooledT[:, h, j * P:(j + 1) * P],
                                    rhs=wsb["wl1_kt"][:, h, :], start=(h == 0),
                                    stop=(h == NH - 1))
                            nc.vector.tensor_add(out=h1[:, j, :], in0=psh[:],
                                                 in1=wsb["bl1_bc"][:])
                    # lnm (local, exact: G*HW elements)
                    s_col = plp.tile([P, 1], f32)
                    nc.vector.tensor_reduce(out=s_col[:],
                                            in_=h1[:].rearrange("p a b -> p (a b)"),
                                            axis=mybir.AxisListType.X, op=OP.add)
                    sq_col = plp.tile([P, 1], f32)
                    sqt2 = plp.tile([P, GT * HW], f32)
                    nc.scalar.activation(out=sqt2[:], in_=h1[:].rearrange("p a b -> p (a b)"),
                                         func=AF.Square, accum_out=sq_col[:])
                    both = plp.tile([P, 2], f32)
                    nc.vector.tensor_copy(out=both[:, 0:1], in_=s_col[:])
                    nc.vector.tensor_copy(out=both[:, 1:2], in_=sq_col[:])
                    tot = part_sum(both, plp)
                    am_a, am_c = ln_stats_to_affine(
                        tot, float(G * HW), wsb["lnmw_bc"], wsb["lnmb_bc"], plp, HW)
                    for j in range(GT):
                        nc.vector.tensor_tensor(out=h1[:, j, :], in0=h1[:, j, :],
                                                in1=am_a[:], op=OP.mult)
                        nc.vector.tensor_add(out=h1[:, j, :], in0=h1[:, j, :], in1=am_c[:])
                    with tc.tile_pool(name="prm", bufs=2) as prp:
                        prelu_blocks(h1[:].rearrange("p a b -> p (a b)"),
                                     h1[:].rearrange("p a b -> p (a b)"), am, prp, f32)
                    # out = h1' @ wl2 + bl2
                    outt = plp.tile([P, GT, OUT], f32)
                    with tc.tile_pool(name="of", bufs=2) as ofp:
                        for j in range(GT):
                            ptp = psc.tile([P, P], f32, space="PSUM", tag="psc_s")
                            nc.tensor.transpose(out=ptp[:], in_=h1[:, j, :],
                                                identity=ident[:])
                            h1t = ofp.tile([P, P], f32, tag="h1t")
                            nc.vector.tensor_copy(out=h1t[:], in_=ptp[:])
                            pso = psc.tile([P, OUT], f32, space="PSUM", tag="psc_s")
                            nc.tensor.matmul(out=pso[:], lhsT=h1t[:], rhs=wsb["wl2"][:],
                                             start=True, stop=True)
                            nc.vector.tensor_add(out=outt[:, j, :], in0=pso[:],
                                                 in1=wsb["bl2_bc"][:, 0:OUT])
                    nc.sync.dma_start(
                        out=out_ext[:].rearrange("(a b) d -> b a d", b=P),
                        in_=outt[:])
                tprep_cm.__exit__(None, None, None)

    nc.compile()
    return nc


# ----------------------------------------------------------------- entry point

def _run(cfg, inputs, use_sim=False):
    import sys
    if '/opt/trn_rl_repo' not in sys.path:
        sys.path.insert(0, '/opt/trn_rl_repo')
    pos = np.asarray(inputs["pos"], np.float32)
    ei = np.asarray(inputs["edge_index"], np.int64)
    batch = np.asarray(inputs["batch"], np.int64)
    meta, core_ins = host_prep(cfg, pos, ei, batch)
    w = _prep_weights(cfg, inputs)
    nc = build_program(cfg, meta, w)
    wnames = ["w1", "b1_cols", "ln1w_cols", "ln1b_cols", "w2_kt", "b2_bc",
              "ln2w_bc", "ln2b_bc", "wl1_kt", "bl1_bc", "lnmw_bc", "lnmb_bc",
              "wl2", "bl2_bc"]
    for ci in range(cfg["NCORES"]):
        for k in wnames:
            core_ins[ci][k] = np.asarray(w[k], np.float32)
    if use_sim:
        from concourse.bass_interp import MultiCoreSim
        sim = MultiCoreSim(nc, cfg["NCORES"])
        for ci in range(cfg["NCORES"]):
            for k, v in core_ins[ci].items():
                sim.cores[ci].tensor(k)[:] = v
        sim.simulate()
        return np.array(sim.cores[0].tensor("out")), None
    from concourse.bass_utils import run_bass_kernel_spmd
    res = run_bass_kernel_spmd(nc, core_ins, list(range(cfg["NCORES"])))
    return res.results[0]["out"], res


def kernel(**inputs):
    out, _ = _run(_cfg_full(), inputs)
    return out


# revision 22
# speedup vs baseline: 2.2841x; 1.0221x over previous
"""GCN (2x GCNConv + graph-layernorm + prelu + mean-pool + MLP head) on 8 trn2 cores.

v2 strategy (dst-sharded graph parallel):
  - nodes and their incoming edges sharded 8 ways by dst; weights replicated.
  - L1: host materializes per-destination padded source lists (pure integer
    indexing / re-layout of the input tensors): pos[src] and deg[src] streams
    laid out [128, NT, 2, K] / [128, NT, K] bf16. Device computes
    rsqrt(deg[src]), multiplies and does one strided reduction -> agg1.
    This removes all per-edge DMA-gather descriptors for layer 1.
  - x1 = prelu(LN(agg1*dinv @ W1 + b1)); LN stats via AllReduce.
  - h2 = x1 @ W2 (bf16 matmuls), scaled by dinv -> fp16 table rows; the table
    row order is quarter-major-permuted so 4 pipelined AllGathers produce
    chunk-contiguous regions addressable with int16 gather indices.
  - L2: per-edge dma_gather of fp16 table rows (optionally two 256B half-row
    gathers sharing one index tile), one-hot scatter matmuls PSUM-chained per
    dst tile across all 4 chunks (supertile-major), y2 kept in bf16.
  - LN2 stats AllReduce; mean-pool via one-hot matmuls; pooled AllReduce;
    MLP head computed redundantly on every core.
All floating point compute happens on device; the host only shards/sorts/pads
integer index metadata and re-lays-out input tensors.
"""

import numpy as np
import ml_dtypes

P = 128
BF16 = ml_dtypes.bfloat16


def _cfg_tiny():
    return dict(
        N=1900, E0=8000, G=128, IN_C=2, HID=256, OUT=16,
        NCORES=8, SH=1024, CT=2, ST=2, HALF_SPLIT=False,
    )


def _cfg_full():
    return dict(
        N=100000, E0=3200000, G=512, IN_C=2, HID=256, OUT=16,
        NCORES=8, SH=12544, CT=16, ST=8, HALF_SPLIT=False,
    )


# ----------------------------------------------------------------- host prep

def _wrap_idx(ix):
    """dma_gather idx layout: idx i -> [i%16 + 16k, i//16] for all k (replicated)."""
    m = ix.reshape(-1, 16).T
    return np.tile(m, (8, 1)).astype(np.int16)


def host_prep(cfg, pos, edge_index, batch):
    c = cfg
    N, E0, G, SH, CT, ST = c["N"], c["E0"], c["G"], c["SH"], c["CT"], c["ST"]
    NC = c["NCORES"]
    NT = SH // P
    NPAD = SH * NC
    NST = (NT + ST - 1) // ST

    # quarter/chunk layout of the h-table (table rows permuted quarter-major)
    base_qt, rem = divmod(NT, 4)
    qtiles = [base_qt + (1 if i < rem else 0) for i in range(4)]
    qstart_t = np.concatenate([[0], np.cumsum(qtiles)])      # tile boundaries
    QH = [qt * P for qt in qtiles]                           # rows per core-quarter
    B = np.concatenate([[0], np.cumsum([NC * q for q in QH])]).astype(np.int64)
    assert B[-1] == NPAD
    assert max(NC * q for q in QH) <= 32768, "chunk exceeds int16 idx range"

    src = np.concatenate([edge_index[0], np.arange(N, dtype=np.int64)])
    dst = np.concatenate([edge_index[1], np.arange(N, dtype=np.int64)])
    E = src.shape[0]
    deg = np.bincount(dst, minlength=NPAD).astype(np.int64)  # in-degree w/ loops

    # ---- L1 per-destination source lists -------------------------------
    K = int(deg.max())
    K = (K + 7) // 8 * 8
    so = np.argsort(dst, kind="stable")
    dsts = dst[so]
    srcs = src[so]
    starts = np.searchsorted(dsts, np.arange(NPAD))
    slot = np.arange(E) - starts[dsts]
    srcmat = np.full((NPAD, K), N, np.int64)                 # sentinel N
    srcmat[dsts, slot] = srcs
    posp = np.vstack([np.asarray(pos, np.float32), np.zeros((1, 2), np.float32)])
    degp = np.ones(N + 1, np.float32)
    degp[:N] = np.maximum(deg[:N], 1)

    # ---- table row permutation ----------------------------------------
    def table_row(n):
        ci = n // SH
        r = n % SH
        t = r // P
        q = np.searchsorted(qstart_t, t, side="right") - 1
        return B[q] + ci * np.asarray(QH)[q] + (r - qstart_t[q] * P)

    trow_all = table_row(srcs)

    # ---- L2 per-core edge cells ---------------------------------------
    bounds = np.searchsorted(dsts, np.arange(NC + 1) * SH)
    NCELL = 4 * NT
    counts = np.zeros((NC, 4, NT), np.int64)
    percore = []
    for ci in range(NC):
        sl = slice(bounds[ci], bounds[ci + 1])
        tr = trow_all[sl]
        dl = dsts[sl] - ci * SH
        tt = dl // P
        qq = np.searchsorted(B, tr, side="right") - 1
        o = np.lexsort((dl, tt, qq))
        tr, dl, tt, qq = tr[o], dl[o], tt[o], qq[o]
        counts[ci] = np.bincount(qq * NT + tt, minlength=NCELL).reshape(4, NT)
        percore.append((tr, dl, tt, qq))

    tiles_cell = (counts.max(axis=0) + P - 1) // P           # [4, NT]
    assert (tiles_cell.sum(axis=0) > 0).all(), "dst tile with no edges"
    TOT = int(tiles_cell.sum())

    # stream cell order: chunk-major (q, t); per-cell first/last tile flags
    tile_meta = []                        # (t, first, last) per stream tile
    CALLS = []                            # (q, ntiles)
    cell_order = []
    for q in range(4):
        k_q = 0
        for t in range(NT):
            n_tiles = int(tiles_cell[q, t])
            for j in range(n_tiles):
                tile_meta.append((t, j == 0, j == n_tiles - 1))
            k_q += n_tiles
            if n_tiles:
                cell_order.append((q, t))
        while k_q > 0:
            k = min(CT, k_q)
            CALLS.append((q, k))
            k_q -= k
    assert len(tile_meta) == TOT

    # per-core idx / dst column streams in stream order
    idx_wrapped, dst_cols = [], []
    for ci in range(NC):
        tr, dl, tt, qq = percore[ci]
        cnt = counts[ci]
        # edge offsets per cell in (q, t) sort order == (s, q, t) stream order?
        # stream order is (s asc, q asc, t asc); sorted edge order is
        # (s, q, t, dl). build per-cell start offsets from the sorted arrays.
        keys = qq * NT + tt
        cell_sizes = np.bincount(keys, minlength=NCELL)
        cell_off = np.concatenate([[0], np.cumsum(cell_sizes)])[:-1]

        idxs = np.zeros(TOT * P, np.int16)
        dcol = np.full(TOT * P, 999.0, np.float16)
        soff = 0
        for (q, t) in cell_order:
            n = int(cnt[q, t])
            e0 = int(cell_off[q * NT + t])
            slots = int(tiles_cell[q, t]) * P
            idxs[soff:soff + n] = (tr[e0:e0 + n] - B[q]).astype(np.int16)
            dcol[soff:soff + n] = (dl[e0:e0 + n] - t * P).astype(np.float16)
            soff += slots
        assert soff == TOT * P

        # wrap idx per call
        parts, off2 = [], 0
        for (_q, k) in CALLS:
            parts.append(_wrap_idx(idxs[off2:off2 + k * P]))
            off2 += k * P
        idx_wrapped.append(np.concatenate(parts, axis=1))    # [128, TOT*8]
        dst_cols.append(dcol.reshape(TOT, P).T.copy())       # [128, TOT]

    # ---- pooling metadata ---------------------------------------------
    gbase = np.zeros(NC, np.int32)
    batch_local = np.full((NC, SH), 999.0, np.float32)
    for ci in range(NC):
        lo, hi = ci * SH, min((ci + 1) * SH, N)
        if lo < N:
            gbase[ci] = batch[lo]
            batch_local[ci, :hi - lo] = (batch[lo:hi] - batch[lo]).astype(np.float32)
            assert batch[hi - 1] - batch[lo] < P - 2, "too many graphs in one shard"
    cnts = np.bincount(batch, minlength=G).astype(np.float32)
    GT = (G + P - 1) // P
    cnt_dev = np.zeros((P, GT), np.float32)
    for g in range(G):
        cnt_dev[g % P, g // P] = cnts[g]

    meta = dict(
        NPAD=NPAD, NT=NT, TOT=TOT, K=K, NST=NST,
        tiles_cell=tiles_cell, CALLS=CALLS, tile_meta=tile_meta,
        qtiles=qtiles, qstart_t=qstart_t, QH=QH, B=B,
        NPADROWS=NPAD - N,
    )

    ins = []
    for ci in range(NC):
        lo, hi = ci * SH, (ci + 1) * SH
        sm = srcmat[lo:hi]                                   # [SH, K]
        pdev = posp[sm].reshape(NT, P, K, 2).transpose(1, 0, 3, 2)
        ddev = degp[sm].reshape(NT, P, K).transpose(1, 0, 2)
        degsh = deg[lo:hi].astype(np.float32).reshape(NT, P).T.copy()
        ins.append(dict(
            pos_st=np.ascontiguousarray(pdev).astype(BF16),
            deg_st=np.ascontiguousarray(ddev).astype(BF16),
            deg_shard=degsh,
            idxs=idx_wrapped[ci], dstc=dst_cols[ci],
            batchl=batch_local[ci].reshape(NT, P).T.astype(np.float32),
            cntg=cnt_dev,
            gbase=np.array([[float(gbase[ci])]], np.float32),
        ))
    return meta, ins


def _prep_weights(cfg, W):
    """Re-layout weights for device (pure replication / transpose-free reshapes)."""
    c = cfg
    HID, OUT, IN_C, G = c["HID"], c["OUT"], c["IN_C"], c["G"]
    NH = HID // P
    w = {}
    w["w1"] = W["w_conv1"].astype(np.float32)                        # [2, 256]
    w["b1_cols"] = np.asarray(W["b_conv1"]).reshape(NH, P).T.copy()
    w["ln1w_cols"] = np.asarray(W["ln1_w"]).reshape(NH, P).T.copy()
    w["ln1b_cols"] = np.asarray(W["ln1_b"]).reshape(NH, P).T.copy()
    w["w2_kt"] = np.ascontiguousarray(
        np.asarray(W["w_conv2"]).reshape(NH, P, HID).transpose(1, 0, 2))
    w["b2_bc"] = np.tile(np.asarray(W["b_conv2"])[None, :], (P, 1)).astype(np.float32)
    w["ln2w_bc"] = np.tile(np.asarray(W["ln2_w"])[None, :], (P, 1)).astype(np.float32)
    w["ln2b_bc"] = np.tile(np.asarray(W["ln2_b"])[None, :], (P, 1)).astype(np.float32)
    w["wl1_kt"] = np.ascontiguousarray(
        np.asarray(W["w_lin1"]).reshape(NH, P, HID // 2).transpose(1, 0, 2))
    w["bl1_bc"] = np.tile(np.asarray(W["b_lin1"])[None, :], (P, 1)).astype(np.float32)
    w["lnmw_bc"] = np.tile(np.asarray(W["lnm_w"])[None, :], (P, 1)).astype(np.float32)
    w["lnmb_bc"] = np.tile(np.asarray(W["lnm_b"])[None, :], (P, 1)).astype(np.float32)
    w["wl2"] = np.asarray(W["w_lin2"]).astype(np.float32)            # [128, 16]
    w["bl2_bc"] = np.tile(np.asarray(W["b_lin2"])[None, :], (P, 1)).astype(np.float32)
    w["a1"] = float(W["a1"]); w["a2"] = float(W["a2"]); w["am"] = float(W["am"])
    return w


# ----------------------------------------------------------------- device build

def build_program(cfg, meta, weights):
    import concourse.bass as bass
    import concourse.mybir as mybir
    import concourse.tile as tile
    from concourse import bacc
    from concourse.masks import make_identity

    c = cfg
    dt = mybir.dt
    N, G, HID, OUT, IN_C = c["N"], c["G"], c["HID"], c["OUT"], c["IN_C"]
    SH, CT, ST = c["SH"], c["CT"], c["ST"]
    NCORES = c["NCORES"]
    HALF = c["HALF_SPLIT"]
    NPAD, NT, TOT, K, NST = meta["NPAD"], meta["NT"], meta["TOT"], meta["K"], meta["NST"]
    CALLS, tile_meta = meta["CALLS"], meta["tile_meta"]
    tiles_cell = meta["tiles_cell"]
    qtiles, qstart_t, QH, B = meta["qtiles"], meta["qstart_t"], meta["QH"], meta["B"]
    NH = HID // P
    GT = (G + P - 1) // P
    NPADROWS = meta["NPADROWS"]
    EPS = 1e-5
    CORE_IDS = list(range(NCORES))
    f32, f16, bf16, i16, i32 = dt.float32, dt.float16, dt.bfloat16, dt.int16, dt.int32
    AF = mybir.ActivationFunctionType
    OP = mybir.AluOpType

    nc = bacc.Bacc("TRN2", debug=False, num_devices=NCORES, num_swdge_queues=4)

    # ---- I/O ----
    pos_st_in = nc.declare_dram_parameter("pos_st", [P, NT, IN_C, K], bf16, isOutput=False)
    deg_st_in = nc.declare_dram_parameter("deg_st", [P, NT, K], bf16, isOutput=False)
    degs_in = nc.declare_dram_parameter("deg_shard", [P, NT], f32, isOutput=False)
    idx_in = nc.declare_dram_parameter("idxs", [P, TOT * 8], i16, isOutput=False)
    dstc_in = nc.declare_dram_parameter("dstc", [P, TOT], f16, isOutput=False)
    batch_in = nc.declare_dram_parameter("batchl", [P, NT], f32, isOutput=False)
    cnt_in = nc.declare_dram_parameter("cntg", [P, GT], f32, isOutput=False)
    gbase_in = nc.declare_dram_parameter("gbase", [1, 1], f32, isOutput=False)
    wt = {}
    wspec = dict(
        w1=[IN_C, HID], b1_cols=[P, NH], ln1w_cols=[P, NH], ln1b_cols=[P, NH],
        w2_kt=[P, NH, HID], b2_bc=[P, HID], ln2w_bc=[P, HID], ln2b_bc=[P, HID],
        wl1_kt=[P, NH, HID // 2], bl1_bc=[P, HID // 2], lnmw_bc=[P, HID // 2],
        lnmb_bc=[P, HID // 2], wl2=[HID // 2, OUT], bl2_bc=[P, OUT],
    )
    for k_, shp in wspec.items():
        wt[k_] = nc.declare_dram_parameter(k_, shp, f32, isOutput=False)
    out_ext = nc.declare_dram_parameter("out", [G, OUT], f32, isOutput=True)
    DBG = c.get("DBG", False)
    if DBG:
        dbg_agg = nc.declare_dram_parameter("dbg_agg", [P, NT * IN_C], f32, isOutput=True)
        dbg_x1 = nc.declare_dram_parameter("dbg_x1", [P, NH * 256], f32, isOutput=True)
        dbg_y1 = nc.declare_dram_parameter("dbg_y1", [P, NH * 256], f32, isOutput=True)
        dbg_st = nc.declare_dram_parameter("dbg_st", [1, 16], f32, isOutput=True)
        dbg_y2 = nc.declare_dram_parameter("dbg_y2", [P, 4 * HID], f32, isOutput=True)

    # ---- internal DRAM ----
    gshard_q = [nc.dram_tensor(f"gshard{q}", [QH[q], HID], f16) for q in range(4)]
    gtab_q = [nc.dram_tensor(f"gtab{q}", [NCORES * QH[q], HID], f16,
                             addr_space="Shared") for q in range(4)]
    st1_in = nc.dram_tensor("st1_in", [1, P], f32)
    st1_out = nc.dram_tensor("st1_out", [1, P], f32, addr_space="Shared")
    st2_in = nc.dram_tensor("st2_in", [1, P], f32)
    st2_out = nc.dram_tensor("st2_out", [1, P], f32, addr_space="Shared")
    POOLR = (GT + 1) * P
    pool_in = nc.dram_tensor("pool_in", [POOLR, HID], f32)
    pool_out = nc.dram_tensor("pool_out", [POOLR, HID], f32, addr_space="Shared")

    a1, a2, am = weights["a1"], weights["a2"], weights["am"]

    with tile.TileContext(nc) as tc:
        with tc.tile_pool(name="persist", bufs=1) as pp, \
             tc.tile_pool(name="psc", bufs=2, space="PSUM") as psc:
            # ---- persistent small tiles ----
            iota_i = pp.tile([P, P], i32)
            nc.gpsimd.iota(iota_i[:], pattern=[[1, P]], base=0, channel_multiplier=0)
            iota_h = pp.tile([P, P], f16)
            nc.vector.tensor_copy(out=iota_h[:], in_=iota_i[:])
            iota_f = pp.tile([P, P], f32)
            nc.vector.tensor_copy(out=iota_f[:], in_=iota_i[:])
            ident = pp.tile([P, P], f32)
            make_identity(nc, ident[:])
            ones_col = pp.tile([P, 1], f32)
            nc.vector.memset(ones_col[:], 1.0)
            ones_row = pp.tile([1, P], f32)
            nc.vector.memset(ones_row[:], 1.0)

            dstc16 = pp.tile([P, TOT], f16)
            nc.sync.dma_start(out=dstc16[:], in_=dstc_in[:])

            wsb = {}
            for k_, shp in wspec.items():
                wsb[k_] = pp.tile(shp, f32, name=f"w_{k_}")
                nc.sync.dma_start(out=wsb[k_][:], in_=wt[k_][:])
            w2b = pp.tile([P, NH, HID], bf16)
            nc.vector.tensor_copy(out=w2b[:], in_=wsb["w2_kt"][:])
            w1b = pp.tile([IN_C, HID], bf16)
            nc.vector.tensor_copy(out=w1b[:], in_=wsb["w1"][:])

            # dinv for this core's dst rows
            deg_s = pp.tile([P, NT], f32)
            nc.sync.dma_start(out=deg_s[:], in_=degs_in[:])
            nc.vector.tensor_scalar(out=deg_s[:], in0=deg_s[:], scalar1=1.0,
                                    scalar2=None, op0=OP.max)
            nc.scalar.sqrt(deg_s[:], deg_s[:])
            dinv_s = pp.tile([P, NT], f32)
            nc.vector.reciprocal(dinv_s[:], deg_s[:])

            # helper: cross-partition sum -> [1,k] sbuf tile
            def part_sum(src_col, w_):
                ps = psc.tile([1, src_col.shape[1]], f32, space="PSUM", tag="psc_s")
                nc.tensor.matmul(out=ps[:], lhsT=ones_col[:], rhs=src_col[:],
                                 start=True, stop=True)
                dstt = w_.tile([1, src_col.shape[1]], f32, tag="psum_scalar")
                nc.vector.tensor_copy(out=dstt[:], in_=ps[:])
                return dstt

            def bcast_col(vals_row, w_):
                k_ = vals_row.shape[1]
                ps = psc.tile([P, k_], f32, space="PSUM", tag="psc_s")
                nc.tensor.matmul(out=ps[:], lhsT=ones_row[:], rhs=vals_row[:],
                                 start=True, stop=True)
                o = w_.tile([P, k_], f32, tag="bcast_col")
                nc.vector.tensor_copy(out=o[:], in_=ps[:])
                return o

            def ln_stats_to_affine(tot_corr, cnt_total, lnw, lnb, stp, wide):
                """tot_corr [1,2] (sum, sumsq) -> affine (a,c) tiles [P, wide]."""
                mean_t = stp.tile([1, 1], f32, tag="mean")
                nc.vector.tensor_scalar(out=mean_t[:], in0=tot_corr[:, 0:1],
                                        scalar1=1.0 / cnt_total, scalar2=None, op0=OP.mult)
                ex2 = stp.tile([1, 1], f32, tag="ex2")
                nc.vector.tensor_scalar(out=ex2[:], in0=tot_corr[:, 1:2],
                                        scalar1=1.0 / cnt_total, scalar2=None, op0=OP.mult)
                m2 = stp.tile([1, 1], f32, tag="m2")
                nc.vector.tensor_tensor(out=m2[:], in0=mean_t[:], in1=mean_t[:], op=OP.mult)
                var = stp.tile([1, 1], f32, tag="var")
                nc.vector.tensor_tensor(out=var[:], in0=ex2[:], in1=m2[:], op=OP.subtract)
                nc.scalar.sqrt(var[:], var[:])
                nc.vector.tensor_scalar(out=var[:], in0=var[:], scalar1=EPS,
                                        scalar2=None, op0=OP.add)
                rstd = stp.tile([1, 1], f32, tag="rstd")
                nc.vector.reciprocal(rstd[:], var[:])
                pack = stp.tile([1, 2], f32, tag="pack")
                nc.vector.tensor_copy(out=pack[:, 0:1], in_=mean_t[:])
                nc.vector.tensor_copy(out=pack[:, 1:2], in_=rstd[:])
                mr = bcast_col(pack, stp)
                a_t = stp.tile([P, wide], f32, tag="a_t")
                c_t = stp.tile([P, wide], f32, tag="c_t")
                nc.vector.tensor_scalar(out=a_t[:], in0=lnw[:],
                                        scalar1=mr[:, 1:2], scalar2=None, op0=OP.mult)
                nc.vector.tensor_scalar(out=c_t[:], in0=a_t[:],
                                        scalar1=mr[:, 0:1], scalar2=None, op0=OP.mult)
                nc.vector.tensor_tensor(out=c_t[:], in0=lnb[:], in1=c_t[:], op=OP.subtract)
                return a_t, c_t


            def prelu_blocks(out_flat, in_flat, alpha, scr_pool, scr_dt):
                cols = in_flat.shape[1]
                BLKP = 4096
                for b0 in range(0, cols, BLKP):
                    b1 = min(b0 + BLKP, cols)
                    scr = scr_pool.tile([P, BLKP], scr_dt, tag="prelu_scr")
                    nc.vector.tensor_scalar(
                        out=scr[:, 0:b1 - b0], in0=in_flat[:, b0:b1],
                        scalar1=0.0, scalar2=float(alpha) - 1.0,
                        op0=OP.min, op1=OP.mult)
                    nc.vector.tensor_tensor(
                        out=out_flat[:, b0:b1], in0=in_flat[:, b0:b1],
                        in1=scr[:, 0:b1 - b0], op=OP.add)

            # =============================== L1 ===============================
            with tc.tile_pool(name="agg1p", bufs=1) as a1p:
                agg1 = a1p.tile([P, NT, IN_C], f32)
                with tc.tile_pool(name="l1s", bufs=1) as l1p, nc.named_scope("L1"):
                    ps_st = l1p.tile([P, NT, IN_C, K], bf16)
                    nc.sync.dma_start(out=ps_st[:], in_=pos_st_in[:])
                    dg_st = l1p.tile([P, NT, K], bf16)
                    nc.sync.dma_start(out=dg_st[:], in_=deg_st_in[:])
                    dsr = l1p.tile([P, NT, K], bf16)
                    with nc.allow_low_precision(reason="bf16 rsqrt of integer degs"):
                        nc.scalar.sqrt(dsr[:].rearrange("p a b -> p (a b)"),
                                       dg_st[:].rearrange("p a b -> p (a b)"))
                        nc.vector.reciprocal(dsr[:].rearrange("p a b -> p (a b)"),
                                             dsr[:].rearrange("p a b -> p (a b)"))
                    _d = dsr[:]
                    d_bc = bass.AP(_d.tensor, _d.offset,
                                   [list(_d.ap[0]), list(_d.ap[1]), [0, IN_C],
                                    list(_d.ap[2])])
                    nc.vector.tensor_tensor(out=ps_st[:], in0=ps_st[:], in1=d_bc,
                                            op=OP.mult)
                    nc.vector.tensor_reduce(
                        out=agg1[:].rearrange("p a b -> p (a b)"),
                        in_=ps_st[:].rearrange("p a b c -> p (a b) c"),
                        axis=mybir.AxisListType.X, op=OP.add)

                if DBG:
                    nc.sync.dma_start(out=dbg_agg[:],
                                      in_=agg1[:].rearrange("p a b -> p (a b)"))
                # ---- x1t = W1 @ (dinv*agg).T + b1 ; layout [128ch, NH, SH] ----
                with tc.tile_pool(name="x1", bufs=1) as x1p, nc.named_scope("mid"):
                    x1t = x1p.tile([P, NH, SH], bf16)
                    p1t_all = x1p.tile([IN_C, SH], bf16)
                    with tc.tile_pool(name="w1w", bufs=4) as wk:
                        for t in range(NT):
                            sc = wk.tile([P, IN_C], f32, tag="sc")
                            nc.vector.tensor_scalar(
                                out=sc[:], in0=agg1[:, t, :],
                                scalar1=dinv_s[:, t:t + 1], scalar2=None, op0=OP.mult)
                            pt = psc.tile([IN_C, P], f32, space="PSUM", tag="psc_s")
                            nc.tensor.transpose(out=pt[:], in_=sc[:], identity=ident[:])
                            nc.vector.tensor_copy(out=p1t_all[:, t * P:(t + 1) * P],
                                                  in_=pt[:])
                        BLK = 512
                        psw_cm = tc.tile_pool(name="psw", bufs=2, space="PSUM")
                        psw = psw_cm.__enter__()
                        for h in range(NH):
                            for b0 in range(0, SH, BLK):
                                b1_ = min(b0 + BLK, SH)
                                psx = psw.tile([P, BLK], f32, space="PSUM", tag="psc_w")
                                nc.tensor.matmul(
                                    out=psx[:, 0:b1_ - b0], lhsT=w1b[:, h * P:(h + 1) * P],
                                    rhs=p1t_all[:, b0:b1_],
                                    start=True, stop=True)
                                with nc.allow_low_precision(reason="bf16 y1"):
                                    nc.vector.tensor_scalar(
                                        out=x1t[:, h, b0:b1_], in0=psx[:, 0:b1_ - b0],
                                        scalar1=wsb["b1_cols"][:, h:h + 1], scalar2=None,
                                        op0=OP.add)

                    psw_cm.__exit__(None, None, None)
                    if DBG:
                        with tc.tile_pool(name="dbg0", bufs=1) as dbp:
                            d0 = dbp.tile([P, NH, 256], f32)
                            nc.vector.tensor_copy(out=d0[:], in_=x1t[:, :, 0:256])
                            nc.sync.dma_start(out=dbg_y1[:],
                                              in_=d0[:].rearrange("p a b -> p (a b)"))
                    # ---- ln1 stats (global over x1) ----
                    with tc.tile_pool(name="stats", bufs=1) as stp:
                        s_col = stp.tile([P, 1], f32)
                        nc.vector.tensor_reduce(out=s_col[:],
                                                in_=x1t[:].rearrange("p a b -> p (a b)"),
                                                axis=mybir.AxisListType.X, op=OP.add)
                        CHK = 2048
                        nchk = (NH * SH + CHK - 1) // CHK
                        sq_cols = stp.tile([P, nchk], f32)
                        sq_scr = stp.tile([P, CHK], f32)
                        x1flat = x1t[:].rearrange("p a b -> p (a b)")
                        for ck in range(nchk):
                            lo, hi = ck * CHK, min((ck + 1) * CHK, NH * SH)
                            nc.scalar.activation(out=sq_scr[:, 0:hi - lo], in_=x1flat[:, lo:hi],
                                                 func=AF.Square, accum_out=sq_cols[:, ck:ck + 1])
                        sq_col = stp.tile([P, 1], f32)
                        nc.vector.tensor_reduce(out=sq_col[:], in_=sq_cols[:],
                                                axis=mybir.AxisListType.X, op=OP.add)
                        both = stp.tile([P, 2], f32)
                        nc.vector.tensor_copy(out=both[:, 0:1], in_=s_col[:])
                        nc.vector.tensor_copy(out=both[:, 1:2], in_=sq_col[:])
                        tot = part_sum(both, stp)            # [1,2] local (sum, sumsq)
                        # b1 pad-row corrections
                        b1s_c = stp.tile([P, 2], f32)
                        nc.vector.tensor_copy(out=b1s_c[:, 0:1], in_=wsb["b1_cols"][:, 0:1])
                        nc.scalar.square(b1s_c[:, 1:2], wsb["b1_cols"][:, 0:1])
                        for h in range(1, NH):
                            nc.vector.tensor_add(out=b1s_c[:, 0:1], in0=b1s_c[:, 0:1],
                                                 in1=wsb["b1_cols"][:, h:h + 1])
                            sqh = stp.tile([P, 1], f32, tag="sqh")
                            nc.scalar.square(sqh[:], wsb["b1_cols"][:, h:h + 1])
                            nc.vector.tensor_add(out=b1s_c[:, 1:2], in0=b1s_c[:, 1:2],
                                                 in1=sqh[:])
                        b1tot = part_sum(b1s_c, stp)
                        arr = stp.tile([1, P], f32)
                        nc.vector.memset(arr[:], 0.0)
                        nc.vector.tensor_copy(out=arr[:, 0:2], in_=tot[:])
                        nc.sync.dma_start(out=st1_in[:], in_=arr[:])
                        nc.gpsimd.collective_compute(
                            "AllReduce", OP.add, replica_groups=[CORE_IDS],
                            ins=[st1_in[:]], outs=[st1_out[:]])
                        arro = stp.tile([1, P], f32)
                        nc.sync.dma_start(out=arro[:], in_=st1_out[:])
                        cor = stp.tile([1, 2], f32)
                        nc.vector.tensor_scalar(out=cor[:], in0=b1tot[:],
                                                scalar1=-float(NPADROWS), scalar2=None,
                                                op0=OP.mult)
                        nc.vector.tensor_add(out=cor[:], in0=cor[:], in1=arro[:, 0:2])
                        if DBG:
                            dstt = stp.tile([1, 16], f32)
                            nc.vector.memset(dstt[:], 0.0)
                            nc.vector.tensor_copy(out=dstt[:, 0:2], in_=tot[:])
                            nc.vector.tensor_copy(out=dstt[:, 2:4], in_=arro[:, 0:2])
                            nc.vector.tensor_copy(out=dstt[:, 4:6], in_=cor[:])
                            nc.sync.dma_start(out=dbg_st[:], in_=dstt[:])
                        acol, ccol = ln_stats_to_affine(
                            cor, float(N * HID), wsb["ln1w_cols"], wsb["ln1b_cols"],
                            stp, NH)
                        with nc.allow_low_precision(reason="bf16 x1 affine"):
                            for h in range(NH):
                                nc.vector.tensor_scalar(
                                    out=x1t[:, h, :], in0=x1t[:, h, :],
                                    scalar1=acol[:, h:h + 1], scalar2=ccol[:, h:h + 1],
                                    op0=OP.mult, op1=OP.add)
                    x1b = x1t
                    with tc.tile_pool(name="pr1", bufs=2) as prp:
                        prelu_blocks(x1b[:].rearrange("p a b -> p (a b)"),
                                     x1t[:].rearrange("p a b -> p (a b)"), a1, prp, bf16)

                    if DBG:
                        with tc.tile_pool(name="dbg1", bufs=1) as dbp:
                            d1 = dbp.tile([P, NH, 256], f32)
                            nc.vector.tensor_copy(out=d1[:], in_=x1b[:, :, 0:256])
                            nc.sync.dma_start(out=dbg_x1[:],
                                              in_=d1[:].rearrange("p a b -> p (a b)"))
                    # ---- h2 per quarter; AllGather per quarter ----
                    with tc.tile_pool(name="h2w", bufs=3) as h2w, \
                         tc.tile_pool(name="ph2", bufs=2, space="PSUM") as ph2:
                        for q in range(4):
                            for t in range(int(qstart_t[q]), int(qstart_t[q + 1])):
                                ps2 = ph2.tile([P, HID], f32, space="PSUM", tag="ph2")
                                for h in range(NH):
                                    nc.tensor.matmul(
                                        out=ps2[:], lhsT=x1b[:, h, t * P:(t + 1) * P],
                                        rhs=w2b[:, h, :], start=(h == 0), stop=(h == NH - 1))
                                g16 = h2w.tile([P, HID], f16, tag="g16")
                                nc.vector.tensor_scalar(
                                    out=g16[:], in0=ps2[:],
                                    scalar1=dinv_s[:, t:t + 1], scalar2=None, op0=OP.mult)
                                tl_ = t - int(qstart_t[q])
                                nc.sync.dma_start(
                                    out=gshard_q[q][tl_ * P:(tl_ + 1) * P, :], in_=g16[:])

            # =============================== L2 ===============================
            with tc.tile_pool(name="y2p", bufs=1) as y2p:
                y2sb = y2p.tile([P, NT, HID], bf16)
                nc.vector.memset(y2sb[:].rearrange("p a b -> p (a b)"), 0.0)
                qtiles_tot = [int(tiles_cell[q].sum()) for q in range(4)]
                QMAX = max(qtiles_tot)
                with tc.tile_pool(name="gbp", bufs=8) as gbp, \
                     tc.tile_pool(name="selp", bufs=6) as selp, \
                     tc.tile_pool(name="idxp", bufs=2) as idxp, \
                     tc.tile_pool(name="paccp", bufs=3, space="PSUM") as paccp, \
                     nc.named_scope("L2agg"):
                    stream_tile = 0
                    off_cols = 0
                    call_i = 0
                    pacc = None
                    cur_q = -1
                    idx_sb = None
                    q_off = 0
                    n_q0 = sum(1 for (q_, _) in CALLS if q_ == 0)
                    n_q1 = sum(1 for (q_, _) in CALLS if q_ == 1)
                    ag_sched = sorted([(0, 0), (min(8, n_q0), 1), (n_q0, 2),
                                       (n_q0 + min(16, n_q1), 3)])
                    ag_i = 0

                    def emit_ag(qa):
                        with nc.named_scope(f"AG{qa}"):
                            nc.gpsimd.collective_compute(
                                "AllGather", OP.bypass, replica_groups=[CORE_IDS],
                                ins=[gshard_q[qa][:]], outs=[gtab_q[qa][:]])

                    for ci_, (q, kk) in enumerate(CALLS):
                        while ag_i < 4 and ag_sched[ag_i][0] <= ci_:
                            emit_ag(ag_sched[ag_i][1])
                            ag_i += 1
                        if q != cur_q:
                            cur_q = q
                            idx_sb = idxp.tile([P, QMAX * 8], i16, tag="idx")
                            nc.sync.dma_start(
                                out=idx_sb[:, 0:qtiles_tot[q] * 8],
                                in_=idx_in[:, off_cols:off_cols + qtiles_tot[q] * 8])
                            q_off = 0
                        idx_t = idx_sb
                        selc = selp.tile([P, CT, P], f16, tag="sel")
                        _i = iota_h[:]
                        _db = dstc16[:, stream_tile:stream_tile + kk]
                        iota_bc = bass.AP(_i.tensor, _i.offset,
                                          [list(_i.ap[0]), [0, kk], list(_i.ap[1])])
                        dst_bc = bass.AP(_db.tensor, _db.offset,
                                         [list(_db.ap[0]), list(_db.ap[1]), [0, P]])
                        nc.vector.tensor_tensor(out=selc[:, 0:kk, :], in0=iota_bc,
                                                in1=dst_bc, op=OP.is_equal)
                        gb = gbp.tile([P, CT, HID], f16, tag="g")
                        nc.gpsimd.dma_gather(
                            out_ap=gb[:, 0:kk, :],
                            idxs_ap=idx_t[:, q_off:q_off + kk * 8],
                            in_ap=gtab_q[q][:],
                            num_idxs=kk * P, num_idxs_reg=kk * P,
                            elem_size=HID, single_packet=False,
                            queue_num=call_i % 4)
                        call_i += 1
                        for j in range(kk):
                            t, first, last = tile_meta[stream_tile]
                            if first:
                                pacc = paccp.tile([P, HID], f32, space="PSUM",
                                                  tag="pacc", name="pacc")
                            nc.tensor.matmul(out=pacc[:], lhsT=selc[:, j, :],
                                             rhs=gb[:, j, :], start=first, stop=last)
                            if last:
                                ev = selp.tile([P, HID], bf16, tag="ev")
                                nc.scalar.activation(out=ev[:], in_=pacc[:],
                                                     func=AF.Copy)
                                with nc.allow_low_precision(reason="bf16 y2 accum"):
                                    nc.vector.tensor_add(out=y2sb[:, t, :],
                                                         in0=y2sb[:, t, :], in1=ev[:])
                            stream_tile += 1
                        off_cols += kk * 8
                        q_off += kk * 8

                    # y2 = dinv * agg + b2 (bulk, in place)
                    _dv = dinv_s[:]
                    dinv_bc = bass.AP(_dv.tensor, _dv.offset,
                                      [list(_dv.ap[0]), list(_dv.ap[1]), [0, HID]])
                    with nc.allow_low_precision(reason="bf16 y2"):
                        nc.vector.tensor_tensor(out=y2sb[:], in0=y2sb[:],
                                                in1=dinv_bc, op=OP.mult)
                        _b2 = wsb["b2_bc"][:]
                        b2_bc3 = bass.AP(_b2.tensor, _b2.offset,
                                         [list(_b2.ap[0]), [0, NT], list(_b2.ap[1])])
                        nc.vector.tensor_tensor(out=y2sb[:], in0=y2sb[:],
                                                in1=b2_bc3, op=OP.add)

                if DBG:
                    with tc.tile_pool(name="dbg2", bufs=1) as dbp:
                        d2 = dbp.tile([P, 4, HID], f32)
                        nc.vector.tensor_copy(out=d2[:], in_=y2sb[:, 0:4, :])
                        nc.sync.dma_start(out=dbg_y2[:],
                                          in_=d2[:].rearrange("p a b -> p (a b)"))
                # ---- ln2 stats + affine + prelu ----
                tprep_cm = tc.tile_pool(name="tprep", bufs=1)
                tprep = tprep_cm.__enter__()
                with tc.tile_pool(name="stats2", bufs=1) as stp:
                    s_col = stp.tile([P, 1], f32)
                    nc.vector.tensor_reduce(out=s_col[:],
                                            in_=y2sb[:].rearrange("p a b -> p (a b)"),
                                            axis=mybir.AxisListType.X, op=OP.add)
                    CHK2 = 2048
                    nchk2 = (NT * HID + CHK2 - 1) // CHK2
                    sq_cols2 = stp.tile([P, nchk2], f32)
                    sq_scr2 = stp.tile([P, CHK2], f32)
                    y2flat = y2sb[:].rearrange("p a b -> p (a b)")
                    for ck in range(nchk2):
                        lo, hi = ck * CHK2, min((ck + 1) * CHK2, NT * HID)
                        nc.scalar.activation(out=sq_scr2[:, 0:hi - lo], in_=y2flat[:, lo:hi],
                                             func=AF.Square, accum_out=sq_cols2[:, ck:ck + 1])
                    sq_col = stp.tile([P, 1], f32)
                    nc.vector.tensor_reduce(out=sq_col[:], in_=sq_cols2[:],
                                            axis=mybir.AxisListType.X, op=OP.add)
                    both = stp.tile([P, 2], f32)
                    nc.vector.tensor_copy(out=both[:, 0:1], in_=s_col[:])
                    nc.vector.tensor_copy(out=both[:, 1:2], in_=sq_col[:])
                    tot = part_sum(both, stp)
                    b2p = stp.tile([1, 2], f32)
                    nc.vector.tensor_reduce(out=b2p[:, 0:1], in_=wsb["b2_bc"][0:1, :],
                                            axis=mybir.AxisListType.X, op=OP.add)
                    b2sq = stp.tile([1, HID], f32)
                    nc.scalar.square(b2sq[:], wsb["b2_bc"][0:1, :])
                    nc.vector.tensor_reduce(out=b2p[:, 1:2], in_=b2sq[:],
                                            axis=mybir.AxisListType.X, op=OP.add)
                    arr = stp.tile([1, P], f32)
                    nc.vector.memset(arr[:], 0.0)
                    nc.vector.tensor_copy(out=arr[:, 0:2], in_=tot[:])
                    nc.sync.dma_start(out=st2_in[:], in_=arr[:])
                    nc.gpsimd.collective_compute(
                        "AllReduce", OP.add, replica_groups=[CORE_IDS],
                        ins=[st2_in[:]], outs=[st2_out[:]])
                    arro = stp.tile([1, P], f32)
                    nc.sync.dma_start(out=arro[:], in_=st2_out[:])
                    # --- tail prep hidden under the AllReduce latency ---
                    batch_sb = tprep.tile([P, NT], f32)
                    nc.sync.dma_start(out=batch_sb[:], in_=batch_in[:])
                    selg_all = tprep.tile([P, NT, P], bf16)
                    for t_ in range(NT):
                        nc.vector.tensor_scalar(
                            out=selg_all[:, t_, :], in0=iota_f[:],
                            scalar1=batch_sb[:, t_:t_ + 1], scalar2=None,
                            op0=OP.is_equal)
                    cor = stp.tile([1, 2], f32)
                    nc.vector.tensor_scalar(out=cor[:], in0=b2p[:],
                                            scalar1=-float(NPADROWS), scalar2=None,
                                            op0=OP.mult)
                    nc.vector.tensor_add(out=cor[:], in0=cor[:], in1=arro[:, 0:2])
                    a_bc, c_bc = ln_stats_to_affine(
                        cor, float(N * HID), wsb["ln2w_bc"], wsb["ln2b_bc"], stp, HID)
                    _ab = a_bc[:]
                    a_bc3 = bass.AP(_ab.tensor, _ab.offset,
                                    [list(_ab.ap[0]), [0, NT], list(_ab.ap[1])])
                    nc.vector.tensor_tensor(out=y2sb[:], in0=y2sb[:], in1=a_bc3,
                                            op=OP.mult)
                    _cb = c_bc[:]
                    c_bc3 = bass.AP(_cb.tensor, _cb.offset,
                                    [list(_cb.ap[0]), [0, NT], list(_cb.ap[1])])
                    nc.vector.tensor_tensor(out=y2sb[:], in0=y2sb[:], in1=c_bc3,
                                            op=OP.add)
                    with tc.tile_pool(name="pr2", bufs=2) as prp:
                        prelu_blocks(y2sb[:].rearrange("p a b -> p (a b)"),
                                     y2sb[:].rearrange("p a b -> p (a b)"), a2, prp, bf16)

                # =========================== pooling ===========================
                with tc.tile_pool(name="poolp", bufs=1) as plp, \
                     tc.tile_pool(name="pps", bufs=1, space="PSUM") as pps, \
                     nc.named_scope("tail"):
                    psg = pps.tile([P, HID], f32, space="PSUM", tag="psg")
                    for t in range(NT):
                        nc.tensor.matmul(out=psg[:], lhsT=selg_all[:, t, :],
                                         rhs=y2sb[:, t, :],
                                         start=(t == 0), stop=(t == NT - 1))
                    partial = plp.tile([P, HID], f32)
                    nc.vector.tensor_copy(out=partial[:], in_=psg[:])
                    # place rows at graph_base via one-hot matmuls; zero the rest
                    gb_sb = plp.tile([1, 1], f32)
                    nc.sync.dma_start(out=gb_sb[:], in_=gbase_in[:])
                    gb_col = bcast_col(gb_sb, plp)
                    pidx_i = plp.tile([P, 1], i32)
                    nc.gpsimd.iota(pidx_i[:], pattern=[[0, 1]], base=0, channel_multiplier=1)
                    pidx = plp.tile([P, 1], f32)
                    nc.vector.tensor_copy(out=pidx[:], in_=pidx_i[:])
                    loc_col = plp.tile([P, 1], f32)
                    nc.vector.tensor_add(out=loc_col[:], in0=pidx[:], in1=gb_col[:])
                    zero_t = plp.tile([P, HID], f32)
                    nc.vector.memset(zero_t[:], 0.0)
                    for j in range(GT + 1):
                        nc.sync.dma_start(out=pool_in[j * P:(j + 1) * P, :], in_=zero_t[:])
                    with tc.tile_pool(name="plc", bufs=2) as plc, \
                         tc.tile_pool(name="ppl", bufs=2, space="PSUM") as ppl:
                        for j in range(GT):
                            sh_col = plc.tile([P, 1], f32, tag="shc")
                            nc.vector.tensor_scalar(out=sh_col[:], in0=loc_col[:],
                                                    scalar1=-float(j * P), scalar2=None,
                                                    op0=OP.add)
                            selj = plc.tile([P, P], f32, tag="selj")
                            nc.vector.tensor_scalar(out=selj[:], in0=iota_f[:],
                                                    scalar1=sh_col[:], scalar2=None,
                                                    op0=OP.is_equal)
                            psj = ppl.tile([P, HID], f32, space="PSUM", tag="psj")
                            nc.tensor.matmul(out=psj[:], lhsT=selj[:], rhs=partial[:],
                                             start=True, stop=True)
                            oj = plc.tile([P, HID], f32, tag="oj")
                            nc.vector.tensor_copy(out=oj[:], in_=psj[:])
                            nc.sync.dma_start(out=pool_in[j * P:(j + 1) * P, :], in_=oj[:])
                    nc.gpsimd.collective_compute(
                        "AllReduce", OP.add, replica_groups=[CORE_IDS],
                        ins=[pool_in[:]], outs=[pool_out[:]])

                    # ---- head (redundant on every core) ----
                    cnt_sb = plp.tile([P, GT], f32)
                    nc.sync.dma_start(out=cnt_sb[:], in_=cnt_in[:])
                    nc.vector.tensor_scalar(out=cnt_sb[:], in0=cnt_sb[:], scalar1=1.0,
                                            scalar2=None, op0=OP.max)
                    rec_sb = plp.tile([P, GT], f32)
                    nc.vector.reciprocal(rec_sb[:], cnt_sb[:])
                    pooled = plp.tile([P, GT, HID], f32)
                    nc.sync.dma_start(
                        out=pooled[:],
                        in_=pool_out[0:G, :].rearrange("(a b) d -> b a d", b=P))
                    for j in range(GT):
                        nc.vector.tensor_scalar(out=pooled[:, j, :], in0=pooled[:, j, :],
                                                scalar1=rec_sb[:, j:j + 1], scalar2=None,
                                                op0=OP.mult)
                    pooledT = plp.tile([P, NH, G], f32)
                    for j in range(GT):
                        for h in range(NH):
                            ptp = psc.tile([P, P], f32, space="PSUM", tag="psc_s")
                            nc.tensor.transpose(
                                out=ptp[:], in_=pooled[:, j, h * P:(h + 1) * P],
                                identity=ident[:])
                            nc.vector.tensor_copy(
                                out=pooledT[:, h, j * P:(j + 1) * P], in_=ptp[:])
                    HW = HID // 2
                    h1 = plp.tile([P, GT, HW], f32)
                    with tc.tile_pool(name="ph1", bufs=2, space="PSUM") as ph1:
                        for j in range(GT):
                            psh = ph1.tile([P, HW], f32, space="PSUM", tag="psh")
                            for h in range(NH):
                                nc.tensor.matmul(
                                    out=psh[:], lhsT=pooledT[:, h, j * P:(j + 1) * P],
                                    rhs=wsb["wl1_kt"][:, h, :], start=(h == 0),
                                    stop=(h == NH - 1))
                            nc.vector.tensor_add(out=h1[:, j, :], in0=psh[:],
                                                 in1=wsb["bl1_bc"][:])
                    # lnm (local, exact: G*HW elements)
                    s_col = plp.tile([P, 1], f32)
                    nc.vector.tensor_reduce(out=s_col[:],
                                            in_=h1[:].rearrange("p a b -> p (a b)"),
                                            axis=mybir.AxisListType.X, op=OP.add)
                    sq_col = plp.tile([P, 1], f32)
                    sqt2 = plp.tile([P, GT * HW], f32)
                    nc.scalar.activation(out=sqt2[:], in_=h1[:].rearrange("p a b -> p (a b)"),
                                         func=AF.Square, accum_out=sq_col[:])
                    both = plp.tile([P, 2], f32)
                    nc.vector.tensor_copy(out=both[:, 0:1], in_=s_col[:])
                    nc.vector.tensor_copy(out=both[:, 1:2], in_=sq_col[:])
                    tot = part_sum(both, plp)
                    am_a, am_c = ln_stats_to_affine(
                        tot, float(G * HW), wsb["lnmw_bc"], wsb["lnmb_bc"], plp, HW)
                    for j in range(GT):
                        nc.vector.tensor_tensor(out=h1[:, j, :], in0=h1[:, j, :],
                                                in1=am_a[:], op=OP.mult)
                        nc.vector.tensor_add(out=h1[:, j, :], in0=h1[:, j, :], in1=am_c[:])
                    with tc.tile_pool(name="prm", bufs=2) as prp:
                        prelu_blocks(h1[:].rearrange("p a b -> p (a b)"),
                                     h1[:].rearrange("p a b -> p (a b)"), am, prp, f32)
                    # out = h1' @ wl2 + bl2
                    outt = plp.tile([P, GT, OUT], f32)
                    with tc.tile_pool(name="of", bufs=2) as ofp:
                        for j in range(GT):
                            ptp = psc.tile([P, P], f32, space="PSUM", tag="psc_s")
                            nc.tensor.transpose(out=ptp[:], in_=h1[:, j, :],
                                                identity=ident[:])
                            h1t = ofp.tile([P, P], f32, tag="h1t")
                            nc.vector.tensor_copy(out=h1t[:], in_=ptp[:])
                            pso = psc.tile([P, OUT], f32, space="PSUM", tag="psc_s")
                            nc.tensor.matmul(out=pso[:], lhsT=h1t[:], rhs=wsb["wl2"][:],
                                             start=True, stop=True)
                            nc.vector.tensor_add(out=outt[:, j, :], in0=pso[:],
                                                 in1=wsb["bl2_bc"][:, 0:OUT])
                    nc.sync.dma_start(
                        out=out_ext[:].rearrange("(a b) d -> b a d", b=P),
                        in_=outt[:])
                tprep_cm.__exit__(None, None, None)

    nc.compile()
    return nc


# ----------------------------------------------------------------- entry point

def _run(cfg, inputs, use_sim=False):
    import sys
    if '/opt/trn_rl_repo' not in sys.path:
        sys.path.insert(0, '/opt/trn_rl_repo')
    pos = np.asarray(inputs["pos"], np.float32)
    ei = np.asarray(inputs["edge_index"], np.int64)
    batch = np.asarray(inputs["batch"], np.int64)
    meta, core_ins = host_prep(cfg, pos, ei, batch)
    w = _prep_weights(cfg, inputs)
    nc = build_program(cfg, meta, w)
    wnames = ["w1", "b1_cols", "ln1w_cols", "ln1b_cols", "w2_kt", "b2_bc",
              "ln2w_bc", "ln2b_bc", "wl1_kt", "bl1_bc", "lnmw_bc", "lnmb_bc",
              "wl2", "bl2_bc"]
    for ci in range(cfg["NCORES"]):
        for k in wnames:
            core_ins[ci][k] = np.asarray(w[k], np.float32)
    if use_sim:
        from concourse.bass_interp import MultiCoreSim
        sim = MultiCoreSim(nc, cfg["NCORES"])
        for ci in range(cfg["NCORES"]):
            for k, v in core_ins[ci].items():
                sim.cores[ci].tensor(k)[:] = v
        sim.simulate()
        return np.array(sim.cores[0].tensor("out")), None
    from concourse.bass_utils import run_bass_kernel_spmd
    res = run_bass_kernel_spmd(nc, core_ins, list(range(cfg["NCORES"])))
    return res.results[0]["out"], res


def kernel(**inputs):
    out, _ = _run(_cfg_full(), inputs)
    return out
